# revision 20
# baseline (speedup 1.0000x reference)
"""CenterLossLayer Trainium2 kernel (8-core SPMD).

Reference computation (B=4096 samples, C=100000 classes, D=128):
    gathered      = centers[labels]                      # via dense one-hot matmul
    delta[c]      = cnt_c * centers[c] - sum_{i: l_i=c} x_i
    new_centers   = centers - 0.5 * delta / (cnt + 1)
    result_i      = ||x_i - gathered_i||^2

Sharding: batch split 8 ways (512 samples/core) for the compute;
classes split 8 ways (12500 rows/core) for the new_centers output.
Each core:
  1. bulk-copies its centers class-shard -> output shard rows [1..CS]
     (dominant memory traffic; row 0 of the output stays all-zero)
  2. computes, for its 512 samples, the FINAL new-center row
     v_i = g - 0.5*(cnt*g - sx)/(cnt+1), where cnt/sx are segment
     counts/sums over the FULL batch obtained with an equality-matrix
     matmul on the tensor engine.  The matmul runs in bf16 with x split
     hi/lo (rhs = [x_hi | x_lo | 1]), giving f32-accurate sums and exact
     counts while using the fast bf16 PE path.
  3. AllGathers the v rows (every core then holds all 4096 rows)
  4. scatter-writes all 4096 rows into its own shard with ONE
     dma_scatter_add: this ucode computes out[idx] = out[row0] + src, so
     with row 0 kept all-zero it is an exact overwrite (verified bitwise
     on HW, including duplicate indices — duplicates carry identical
     bits).  Out-of-shard rows are clamped to a trash row.
"""

import os
import sys

import numpy as np

for _p in ("/opt/trn_rl_repo", "/root/.axon_site/_ro/trn_rl_repo"):
    if os.path.isdir(_p) and _p not in sys.path:
        sys.path.insert(0, _p)

import concourse.bass as bass
import concourse.bacc as bacc
import concourse.mybir as mybir
import concourse.tile as tile
from concourse import bass_utils

import ml_dtypes

B, C, D = 4096, 100000, 128
NCORES = 8
BS = B // NCORES          # 512 samples per core
CS = C // NCORES          # 12500 classes per core
TRASH = CS + 1            # trash row (after the zero row + CS class rows)
NB = B // 128             # 32 all-batch chunks
NM = BS // 128            # 4 own-batch chunks
XW = 2 * D + 1            # [x_hi | x_lo | ones] columns

F32 = mybir.dt.float32
BF16 = mybir.dt.bfloat16
I32 = mybir.dt.int32
I16 = mybir.dt.int16

_NC_CACHE = None


def _build_nc(dbg=False):
    nc = bacc.Bacc(
        "TRN2", target_bir_lowering=False, debug=False, num_devices=NCORES
    )

    x_own = nc.dram_tensor("x_own", [BS, D], F32, kind="ExternalInput")
    xcat = nc.dram_tensor("xcat", [B, XW], BF16, kind="ExternalInput")
    lab_own = nc.dram_tensor("lab_own", [1, BS], F32, kind="ExternalInput")
    lab_all_pt = nc.dram_tensor("lab_all_pt", [128, NB], F32, kind="ExternalInput")
    gidx_pt = nc.dram_tensor("gidx_pt", [128, NM], I32, kind="ExternalInput")
    valid_pt = nc.dram_tensor("valid_pt", [128, NM], F32, kind="ExternalInput")
    sc16 = nc.dram_tensor("sc16", [128, B // 16], I16, kind="ExternalInput")
    centers_all = nc.dram_tensor("centers_all", [C, D], F32, kind="ExternalInput")
    centers_shard = nc.dram_tensor("centers_shard", [CS, D], F32, kind="ExternalInput")

    result_own = nc.dram_tensor("result_own", [BS, 1], F32, kind="ExternalOutput")
    # row 0: always zero (scatter base); rows 1..CS: classes; row CS+1: trash
    out_shard = nc.dram_tensor("new_centers_shard", [CS + 2, D], F32, kind="ExternalOutput")

    v_own_d = nc.dram_tensor("v_own_d", [BS, D], F32)
    v_all_d = nc.dram_tensor("v_all_d", [B, D], F32, addr_space="Shared")

    if dbg:
        vall_dump = nc.dram_tensor("vall_dump", [B, D], F32, kind="ExternalOutput")

    with tile.TileContext(nc) as tc:
        with (
            tc.tile_pool(name="sbuf", bufs=1) as cpool,
            tc.tile_pool(name="ebuf", bufs=4) as epool,
            tc.tile_pool(name="psum", bufs=1, space="PSUM") as ppool,
        ):
            # --- stage inputs in SBUF (sync ring) ---
            lab_all_sb = cpool.tile([128, NB], F32)
            nc.sync.dma_start(out=lab_all_sb[:], in_=lab_all_pt[:, :])
            lab_own_sb = cpool.tile([1, BS], F32)
            nc.sync.dma_start(out=lab_own_sb[:], in_=lab_own[:, :])
            gidx_sb = cpool.tile([128, NM], I32)
            nc.sync.dma_start(out=gidx_sb[:], in_=gidx_pt[:, :])
            valid_sb = cpool.tile([128, NM], F32)
            nc.sync.dma_start(out=valid_sb[:], in_=valid_pt[:, :])
            sc16_sb = cpool.tile([128, B // 16], I16)
            nc.sync.dma_start(out=sc16_sb[:], in_=sc16[:, :])
            xc_sb = cpool.tile([128, NB, XW], BF16)
            nc.sync.dma_start(
                out=xc_sb[:], in_=xcat.rearrange("(t p) c -> p t c", p=128)
            )
            x_own_sb = cpool.tile([128, NM, D], F32)
            nc.sync.dma_start(
                out=x_own_sb[:], in_=x_own.rearrange("(t p) d -> p t d", p=128)
            )

            # --- bulk copy centers shard -> output rows [1..CS] (scalar ring,
            # so it doesn't queue in front of the input loads above) ---
            n_pieces = 4
            rows = CS // n_pieces
            for i in range(n_pieces):
                nc.scalar.dma_start(
                    out=out_shard[1 + i * rows : 1 + (i + 1) * rows, :],
                    in_=centers_shard[i * rows : (i + 1) * rows, :],
                )

            # broadcast own labels across partitions: ones^T @ lab_own
            ones1 = cpool.tile([1, 128], F32)
            nc.vector.memset(ones1[:], 1.0)
            bc_psum = ppool.tile([128, BS], F32)
            nc.tensor.matmul(
                bc_psum[:], lhsT=ones1[:], rhs=lab_own_sb[:], start=True, stop=True
            )
            bcast_sb = cpool.tile([128, BS], F32)
            nc.vector.tensor_copy(bcast_sb[:], bc_psum[:])

            # gather own centers rows g = centers[labels_own]
            g_sb = cpool.tile([128, NM, D], F32)
            for m in range(NM):
                nc.gpsimd.indirect_dma_start(
                    out=g_sb[:, m, :],
                    out_offset=None,
                    in_=centers_all[:, :],
                    in_offset=bass.IndirectOffsetOnAxis(ap=gidx_sb[:, m : m + 1], axis=0),
                )

            # --- equality-matrix chunks + segment-sum matmuls (bf16) ---
            # psum[m][s, :] = [sum_x_hi (D) | sum_x_lo (D) | count (1)]
            ps = []
            for m in range(NM):
                ps.append(
                    ppool.tile([128, XW], F32, tag=f"ps{m}", name=f"ps{m}")
                )
            for j in range(NB):
                e_t = epool.tile([128, BS], BF16, tag="e", name="e_t")
                eng = nc.vector if (j % 2 == 0) else nc.gpsimd
                eng.tensor_scalar(
                    out=e_t[:],
                    in0=bcast_sb[:],
                    scalar1=lab_all_sb[:, j : j + 1],
                    scalar2=None,
                    op0=mybir.AluOpType.is_equal,
                )
                for m in range(NM):
                    nc.tensor.matmul(
                        ps[m][:],
                        lhsT=e_t[:, m * 128 : (m + 1) * 128],
                        rhs=xc_sb[:, j, :],
                        start=(j == 0),
                        stop=(j == NB - 1),
                    )

            # --- per-chunk epilogue: result + final new-center rows v ---
            v_sb = cpool.tile([128, NM, D], F32)
            for m in range(NM):
                psb = cpool.tile([128, XW], F32, tag=f"psb{m}", name=f"psb{m}")
                nc.vector.tensor_copy(psb[:], ps[m][:])
                sx = cpool.tile([128, D], F32, tag=f"sx{m}", name=f"sx{m}")
                nc.vector.tensor_tensor(
                    out=sx[:], in0=psb[:, 0:D], in1=psb[:, D : 2 * D],
                    op=mybir.AluOpType.add,
                )
                cnt = psb[:, 2 * D : XW]

                gm = cpool.tile([128, D], F32, tag=f"gm{m}", name=f"gm{m}")
                nc.vector.tensor_scalar(
                    out=gm[:],
                    in0=g_sb[:, m, :],
                    scalar1=valid_sb[:, m : m + 1],
                    scalar2=None,
                    op0=mybir.AluOpType.mult,
                )

                diff = cpool.tile([128, D], F32, tag=f"diff{m}", name=f"diff{m}")
                nc.vector.tensor_tensor(
                    out=diff[:], in0=x_own_sb[:, m, :], in1=gm[:],
                    op=mybir.AluOpType.subtract,
                )
                sq = cpool.tile([128, D], F32, tag=f"sq{m}", name=f"sq{m}")
                nc.vector.tensor_tensor(
                    out=sq[:], in0=diff[:], in1=diff[:], op=mybir.AluOpType.mult
                )
                res = cpool.tile([128, 1], F32, tag=f"res{m}", name=f"res{m}")
                nc.vector.tensor_reduce(
                    out=res[:], in_=sq[:], axis=mybir.AxisListType.X,
                    op=mybir.AluOpType.add,
                )
                nc.sync.dma_start(
                    out=result_own[m * 128 : (m + 1) * 128, :], in_=res[:]
                )

                # v = g + 0.5*(sx - cnt*g)/(cnt+1)   (the FINAL new-center row)
                den = cpool.tile([128, 1], F32, tag=f"den{m}", name=f"den{m}")
                nc.vector.tensor_scalar(
                    out=den[:], in0=cnt[:], scalar1=1.0, scalar2=None,
                    op0=mybir.AluOpType.add,
                )
                rec = cpool.tile([128, 1], F32, tag=f"rec{m}", name=f"rec{m}")
                nc.vector.reciprocal(rec[:], den[:])
                rec2 = cpool.tile([128, 1], F32, tag=f"rec2{m}", name=f"rec2{m}")
                nc.vector.tensor_scalar(
                    out=rec2[:], in0=rec[:], scalar1=0.5, scalar2=None,
                    op0=mybir.AluOpType.mult,
                )
                t1 = cpool.tile([128, D], F32, tag=f"t1{m}", name=f"t1{m}")
                nc.vector.tensor_scalar(
                    out=t1[:], in0=gm[:], scalar1=cnt[:], scalar2=None,
                    op0=mybir.AluOpType.mult,
                )
                t2 = cpool.tile([128, D], F32, tag=f"t2{m}", name=f"t2{m}")
                nc.vector.tensor_tensor(
                    out=t2[:], in0=sx[:], in1=t1[:],
                    op=mybir.AluOpType.subtract,
                )
                t3 = cpool.tile([128, D], F32, tag=f"t3{m}", name=f"t3{m}")
                nc.vector.tensor_scalar(
                    out=t3[:], in0=t2[:], scalar1=rec2[:], scalar2=None,
                    op0=mybir.AluOpType.mult,
                )
                nc.vector.tensor_tensor(
                    out=v_sb[:, m, :], in0=gm[:], in1=t3[:],
                    op=mybir.AluOpType.add,
                )

            # --- exchange v rows: every core ends up with all B rows ---
            nc.sync.dma_start(
                out=v_own_d.rearrange("(t p) d -> p t d", p=128), in_=v_sb[:]
            )
            nc.gpsimd.collective_compute(
                "AllGather",
                mybir.AluOpType.bypass,
                replica_groups=[list(range(NCORES))],
                ins=[v_own_d[:, :].opt()],
                outs=[v_all_d[:, :].opt()],
            )

            # --- scatter all B rows into own shard in ONE op ---
            v_all_sb = cpool.tile([128, NB, D], F32)
            nc.sync.dma_start(
                out=v_all_sb[:], in_=v_all_d.rearrange("(t p) d -> p t d", p=128)
            )
            if dbg:
                nc.sync.dma_start(
                    out=vall_dump.rearrange("(t p) d -> p t d", p=128),
                    in_=v_all_sb[:],
                )
            # out[idx] = out[row 0] + src on this ucode; row 0 is all-zero, so
            # this is an exact overwrite (HW-verified bitwise, incl duplicates)
            nc.gpsimd.dma_scatter_add(
                out_shard[:, :],
                v_all_sb[:],
                sc16_sb[:],
                B,
                B,
                D,
            )

    nc.compile()
    return nc


def _get_nc():
    global _NC_CACHE
    if _NC_CACHE is None:
        _NC_CACHE = _build_nc()
    return _NC_CACHE


def _make_in_maps(x, onehot, centers):
    x = np.ascontiguousarray(np.asarray(x, dtype=np.float32))
    centers = np.ascontiguousarray(np.asarray(centers, dtype=np.float32))
    onehot = np.asarray(onehot)

    labels = np.argmax(onehot, axis=1).astype(np.int64)
    valid = np.asarray(onehot[np.arange(B), labels]) > 0.5
    labf = np.where(valid, labels.astype(np.float32), np.float32(-1.0)).astype(
        np.float32
    )
    gidx = np.where(valid, labels, 0).astype(np.int32)
    validf = valid.astype(np.float32)

    lab_all_pt = np.ascontiguousarray(labf.reshape(NB, 128).T)

    # x split hi/lo in bf16 + ones column: [B, 2D+1]
    xh = x.astype(ml_dtypes.bfloat16)
    xl = (x - xh.astype(np.float32)).astype(ml_dtypes.bfloat16)
    xcat = np.ones((B, XW), ml_dtypes.bfloat16)
    xcat[:, 0:D] = xh
    xcat[:, D : 2 * D] = xl

    in_maps = []
    for k in range(NCORES):
        sl = slice(k * BS, (k + 1) * BS)
        loc = labels - k * CS
        ok = valid & (loc >= 0) & (loc < CS)
        # +1: row 0 of the output shard is the reserved zero row
        loc16 = np.where(ok, loc + 1, TRASH).astype(np.int16)
        sc = np.zeros((128, B // 16), np.int16)
        sc[:16, :] = loc16.reshape(B // 16, 16).T
        in_maps.append(
            {
                "x_own": x[sl],
                "xcat": xcat,
                "lab_own": np.ascontiguousarray(labf[sl].reshape(1, BS)),
                "lab_all_pt": lab_all_pt,
                "gidx_pt": np.ascontiguousarray(gidx[sl].reshape(NM, 128).T),
                "valid_pt": np.ascontiguousarray(validf[sl].reshape(NM, 128).T),
                "sc16": sc,
                "centers_all": centers,
                "centers_shard": centers[k * CS : (k + 1) * CS],
            }
        )
    return in_maps


def _assemble(results):
    result = np.concatenate(
        [results[k]["result_own"] for k in range(NCORES)], axis=0
    ).astype(np.float32)
    new_centers = np.concatenate(
        [results[k]["new_centers_shard"][1 : CS + 1] for k in range(NCORES)], axis=0
    ).astype(np.float32)
    return result, new_centers


def run_traced(x, onehot, centers, trace=True):
    """Run on hardware with NTFF profiling; returns ((result, new_centers), exec_ns)."""
    nc = _get_nc()
    in_maps = _make_in_maps(x, onehot, centers)
    res = bass_utils.run_bass_kernel_spmd(
        nc, in_maps, list(range(NCORES)), trace=trace
    )
    return _assemble(res.results), res.exec_time_ns


def kernel(x, onehot, centers):
    nc = _get_nc()
    in_maps = _make_in_maps(x, onehot, centers)
    res = bass_utils.run_bass_kernel_spmd(nc, in_maps, list(range(NCORES)))
    return _assemble(res.results)


# revision 22
# speedup vs baseline: 1.3111x; 1.3111x over previous
"""CenterLossLayer Trainium2 kernel (8-core SPMD).

Reference computation (B=4096 samples, C=100000 classes, D=128):
    gathered      = centers[labels]                      # via dense one-hot matmul
    delta[c]      = cnt_c * centers[c] - sum_{i: l_i=c} x_i
    new_centers   = centers - 0.5 * delta / (cnt + 1)
    result_i      = ||x_i - gathered_i||^2

Sharding: batch split 8 ways (512 samples/core) for the compute;
classes split 8 ways (12500 rows/core) for the new_centers output.
Each core:
  1. bulk-copies its centers class-shard -> output shard rows [1..CS]
     (dominant memory traffic; row 0 of the output stays all-zero)
  2. computes, for its 512 samples, the FINAL new-center row
     v_i = g - 0.5*(cnt*g - sx)/(cnt+1), where cnt/sx are segment
     counts/sums over the FULL batch obtained with an equality-matrix
     matmul on the tensor engine.  The matmul runs in bf16 with x split
     hi/lo (rhs = [x_hi | x_lo | 1]), giving f32-accurate sums and exact
     counts while using the fast bf16 PE path.
  3. AllGathers the v rows (every core then holds all 4096 rows)
  4. scatter-writes all 4096 rows into its own shard with ONE
     dma_scatter_add: this ucode computes out[idx] = out[row0] + src, so
     with row 0 kept all-zero it is an exact overwrite (verified bitwise
     on HW, including duplicate indices — duplicates carry identical
     bits).  Out-of-shard rows are clamped to a trash row.
"""

import os
import sys

import numpy as np

for _p in ("/opt/trn_rl_repo", "/root/.axon_site/_ro/trn_rl_repo"):
    if os.path.isdir(_p) and _p not in sys.path:
        sys.path.insert(0, _p)

import concourse.bass as bass
import concourse.bacc as bacc
import concourse.mybir as mybir
import concourse.tile as tile
from concourse import bass_utils

import ml_dtypes

B, C, D = 4096, 100000, 128
NCORES = 8
BS = B // NCORES          # 512 samples per core
CS = C // NCORES          # 12500 classes per core
TRASH = CS + 1            # trash row (after the zero row + CS class rows)
NB = B // 128             # 32 all-batch chunks
NM = BS // 128            # 4 own-batch chunks
XW = 2 * D + 1            # [x_hi | x_lo | ones] columns

F32 = mybir.dt.float32
BF16 = mybir.dt.bfloat16
I32 = mybir.dt.int32
I16 = mybir.dt.int16

_NC_CACHE = None


def _build_nc(dbg=False):
    nc = bacc.Bacc(
        "TRN2", target_bir_lowering=False, debug=False, num_devices=NCORES
    )

    x_own = nc.dram_tensor("x_own", [BS, D], F32, kind="ExternalInput")
    xcat = nc.dram_tensor("xcat", [B, XW], BF16, kind="ExternalInput")
    lab_own = nc.dram_tensor("lab_own", [1, BS], F32, kind="ExternalInput")
    lab_all_pt = nc.dram_tensor("lab_all_pt", [128, NB], F32, kind="ExternalInput")
    gidx_pt = nc.dram_tensor("gidx_pt", [128, NM], I32, kind="ExternalInput")
    valid_pt = nc.dram_tensor("valid_pt", [128, NM], F32, kind="ExternalInput")
    sc16 = nc.dram_tensor("sc16", [128, B // 16], I16, kind="ExternalInput")
    centers_all = nc.dram_tensor("centers_all", [C, D], F32, kind="ExternalInput")
    centers_shard = nc.dram_tensor("centers_shard", [CS, D], F32, kind="ExternalInput")

    result_own = nc.dram_tensor("result_own", [BS, 1], F32, kind="ExternalOutput")
    # row 0: always zero (scatter base); rows 1..CS: classes; row CS+1: trash
    out_shard = nc.dram_tensor("new_centers_shard", [CS + 2, D], F32, kind="ExternalOutput")

    v_own_d = nc.dram_tensor("v_own_d", [BS, D], F32)
    v_all_d = nc.dram_tensor("v_all_d", [B, D], F32, addr_space="Shared")

    if dbg:
        vall_dump = nc.dram_tensor("vall_dump", [B, D], F32, kind="ExternalOutput")

    with tile.TileContext(nc) as tc:
        with (
            tc.tile_pool(name="sbuf", bufs=1) as cpool,
            tc.tile_pool(name="ebuf", bufs=4) as epool,
            tc.tile_pool(name="psum", bufs=1, space="PSUM") as ppool,
        ):
            # --- stage inputs in SBUF (sync ring) ---
            lab_all_sb = cpool.tile([128, NB], F32)
            nc.sync.dma_start(out=lab_all_sb[:], in_=lab_all_pt[:, :])
            lab_own_sb = cpool.tile([1, BS], F32)
            nc.sync.dma_start(out=lab_own_sb[:], in_=lab_own[:, :])
            gidx_sb = cpool.tile([128, NM], I32)
            nc.sync.dma_start(out=gidx_sb[:], in_=gidx_pt[:, :])
            valid_sb = cpool.tile([128, NM], F32)
            nc.sync.dma_start(out=valid_sb[:], in_=valid_pt[:, :])
            sc16_sb = cpool.tile([128, B // 16], I16)
            nc.sync.dma_start(out=sc16_sb[:], in_=sc16[:, :])
            xc_sb = cpool.tile([128, NB, XW], BF16)
            nc.sync.dma_start(
                out=xc_sb[:], in_=xcat.rearrange("(t p) c -> p t c", p=128)
            )
            x_own_sb = cpool.tile([128, NM, D], F32)
            nc.sync.dma_start(
                out=x_own_sb[:], in_=x_own.rearrange("(t p) d -> p t d", p=128)
            )

            # --- bulk copy centers shard -> output rows [1..CS] (scalar ring,
            # so it doesn't queue in front of the input loads above) ---
            n_pieces = 4
            rows = CS // n_pieces
            for i in range(n_pieces):
                nc.scalar.dma_start(
                    out=out_shard[1 + i * rows : 1 + (i + 1) * rows, :],
                    in_=centers_shard[i * rows : (i + 1) * rows, :],
                )

            # broadcast own labels across partitions: ones^T @ lab_own
            ones1 = cpool.tile([1, 128], F32)
            nc.vector.memset(ones1[:], 1.0)
            bc_psum = ppool.tile([128, BS], F32)
            nc.tensor.matmul(
                bc_psum[:], lhsT=ones1[:], rhs=lab_own_sb[:], start=True, stop=True
            )
            bcast_sb = cpool.tile([128, BS], F32)
            nc.vector.tensor_copy(bcast_sb[:], bc_psum[:])

            # gather own centers rows g = centers[labels_own]
            g_sb = cpool.tile([128, NM, D], F32)
            for m in range(NM):
                nc.gpsimd.indirect_dma_start(
                    out=g_sb[:, m, :],
                    out_offset=None,
                    in_=centers_all[:, :],
                    in_offset=bass.IndirectOffsetOnAxis(ap=gidx_sb[:, m : m + 1], axis=0),
                )

            # --- equality-matrix chunks + segment-sum matmuls (bf16) ---
            # psum[m][s, :] = [sum_x_hi (D) | sum_x_lo (D) | count (1)]
            ps = []
            for m in range(NM):
                ps.append(
                    ppool.tile([128, XW], F32, tag=f"ps{m}", name=f"ps{m}")
                )
            for j in range(NB):
                # f32 compare on DVE (fast path), f32->bf16 cast on Scalar/ACT
                # (mixed-dtype tensor_scalar is ~8x slower on DVE; ACT is idle)
                e_f = epool.tile([128, BS], F32, tag="ef", name="e_f")
                nc.vector.tensor_scalar(
                    out=e_f[:],
                    in0=bcast_sb[:],
                    scalar1=lab_all_sb[:, j : j + 1],
                    scalar2=None,
                    op0=mybir.AluOpType.is_equal,
                )
                e_t = epool.tile([128, BS], BF16, tag="e", name="e_t")
                nc.scalar.copy(out=e_t[:], in_=e_f[:])
                for m in range(NM):
                    nc.tensor.matmul(
                        ps[m][:],
                        lhsT=e_t[:, m * 128 : (m + 1) * 128],
                        rhs=xc_sb[:, j, :],
                        start=(j == 0),
                        stop=(j == NB - 1),
                    )

            # --- per-chunk epilogue: result + final new-center rows v ---
            v_sb = cpool.tile([128, NM, D], F32)
            for m in range(NM):
                psb = cpool.tile([128, XW], F32, tag=f"psb{m}", name=f"psb{m}")
                nc.vector.tensor_copy(psb[:], ps[m][:])
                sx = cpool.tile([128, D], F32, tag=f"sx{m}", name=f"sx{m}")
                nc.vector.tensor_tensor(
                    out=sx[:], in0=psb[:, 0:D], in1=psb[:, D : 2 * D],
                    op=mybir.AluOpType.add,
                )
                cnt = psb[:, 2 * D : XW]

                gm = cpool.tile([128, D], F32, tag=f"gm{m}", name=f"gm{m}")
                nc.vector.tensor_scalar(
                    out=gm[:],
                    in0=g_sb[:, m, :],
                    scalar1=valid_sb[:, m : m + 1],
                    scalar2=None,
                    op0=mybir.AluOpType.mult,
                )

                diff = cpool.tile([128, D], F32, tag=f"diff{m}", name=f"diff{m}")
                nc.vector.tensor_tensor(
                    out=diff[:], in0=x_own_sb[:, m, :], in1=gm[:],
                    op=mybir.AluOpType.subtract,
                )
                sq = cpool.tile([128, D], F32, tag=f"sq{m}", name=f"sq{m}")
                nc.vector.tensor_tensor(
                    out=sq[:], in0=diff[:], in1=diff[:], op=mybir.AluOpType.mult
                )
                res = cpool.tile([128, 1], F32, tag=f"res{m}", name=f"res{m}")
                nc.vector.tensor_reduce(
                    out=res[:], in_=sq[:], axis=mybir.AxisListType.X,
                    op=mybir.AluOpType.add,
                )
                nc.sync.dma_start(
                    out=result_own[m * 128 : (m + 1) * 128, :], in_=res[:]
                )

                # v = g + 0.5*(sx - cnt*g)/(cnt+1)   (the FINAL new-center row)
                den = cpool.tile([128, 1], F32, tag=f"den{m}", name=f"den{m}")
                nc.vector.tensor_scalar(
                    out=den[:], in0=cnt[:], scalar1=1.0, scalar2=None,
                    op0=mybir.AluOpType.add,
                )
                rec = cpool.tile([128, 1], F32, tag=f"rec{m}", name=f"rec{m}")
                nc.vector.reciprocal(rec[:], den[:])
                rec2 = cpool.tile([128, 1], F32, tag=f"rec2{m}", name=f"rec2{m}")
                nc.vector.tensor_scalar(
                    out=rec2[:], in0=rec[:], scalar1=0.5, scalar2=None,
                    op0=mybir.AluOpType.mult,
                )
                t1 = cpool.tile([128, D], F32, tag=f"t1{m}", name=f"t1{m}")
                nc.vector.tensor_scalar(
                    out=t1[:], in0=gm[:], scalar1=cnt[:], scalar2=None,
                    op0=mybir.AluOpType.mult,
                )
                t2 = cpool.tile([128, D], F32, tag=f"t2{m}", name=f"t2{m}")
                nc.vector.tensor_tensor(
                    out=t2[:], in0=sx[:], in1=t1[:],
                    op=mybir.AluOpType.subtract,
                )
                t3 = cpool.tile([128, D], F32, tag=f"t3{m}", name=f"t3{m}")
                nc.vector.tensor_scalar(
                    out=t3[:], in0=t2[:], scalar1=rec2[:], scalar2=None,
                    op0=mybir.AluOpType.mult,
                )
                nc.vector.tensor_tensor(
                    out=v_sb[:, m, :], in0=gm[:], in1=t3[:],
                    op=mybir.AluOpType.add,
                )

            # --- exchange v rows: every core ends up with all B rows ---
            nc.sync.dma_start(
                out=v_own_d.rearrange("(t p) d -> p t d", p=128), in_=v_sb[:]
            )
            nc.gpsimd.collective_compute(
                "AllGather",
                mybir.AluOpType.bypass,
                replica_groups=[list(range(NCORES))],
                ins=[v_own_d[:, :].opt()],
                outs=[v_all_d[:, :].opt()],
            )

            # --- scatter all B rows into own shard in ONE op ---
            v_all_sb = cpool.tile([128, NB, D], F32)
            nc.sync.dma_start(
                out=v_all_sb[:], in_=v_all_d.rearrange("(t p) d -> p t d", p=128)
            )
            if dbg:
                nc.sync.dma_start(
                    out=vall_dump.rearrange("(t p) d -> p t d", p=128),
                    in_=v_all_sb[:],
                )
            # out[idx] = out[row 0] + src on this ucode; row 0 is all-zero, so
            # this is an exact overwrite (HW-verified bitwise, incl duplicates)
            nc.gpsimd.dma_scatter_add(
                out_shard[:, :],
                v_all_sb[:],
                sc16_sb[:],
                B,
                B,
                D,
                single_packet=False,
            )

    nc.compile()
    return nc


def _get_nc():
    global _NC_CACHE
    if _NC_CACHE is None:
        _NC_CACHE = _build_nc()
    return _NC_CACHE


def _make_in_maps(x, onehot, centers):
    x = np.ascontiguousarray(np.asarray(x, dtype=np.float32))
    centers = np.ascontiguousarray(np.asarray(centers, dtype=np.float32))
    onehot = np.asarray(onehot)

    labels = np.argmax(onehot, axis=1).astype(np.int64)
    valid = np.asarray(onehot[np.arange(B), labels]) > 0.5
    labf = np.where(valid, labels.astype(np.float32), np.float32(-1.0)).astype(
        np.float32
    )
    gidx = np.where(valid, labels, 0).astype(np.int32)
    validf = valid.astype(np.float32)

    lab_all_pt = np.ascontiguousarray(labf.reshape(NB, 128).T)

    # x split hi/lo in bf16 + ones column: [B, 2D+1]
    xh = x.astype(ml_dtypes.bfloat16)
    xl = (x - xh.astype(np.float32)).astype(ml_dtypes.bfloat16)
    xcat = np.ones((B, XW), ml_dtypes.bfloat16)
    xcat[:, 0:D] = xh
    xcat[:, D : 2 * D] = xl

    in_maps = []
    for k in range(NCORES):
        sl = slice(k * BS, (k + 1) * BS)
        loc = labels - k * CS
        ok = valid & (loc >= 0) & (loc < CS)
        # +1: row 0 of the output shard is the reserved zero row
        loc16 = np.where(ok, loc + 1, TRASH).astype(np.int16)
        sc = np.zeros((128, B // 16), np.int16)
        sc[:16, :] = loc16.reshape(B // 16, 16).T
        in_maps.append(
            {
                "x_own": x[sl],
                "xcat": xcat,
                "lab_own": np.ascontiguousarray(labf[sl].reshape(1, BS)),
                "lab_all_pt": lab_all_pt,
                "gidx_pt": np.ascontiguousarray(gidx[sl].reshape(NM, 128).T),
                "valid_pt": np.ascontiguousarray(validf[sl].reshape(NM, 128).T),
                "sc16": sc,
                "centers_all": centers,
                "centers_shard": centers[k * CS : (k + 1) * CS],
            }
        )
    return in_maps


def _assemble(results):
    result = np.concatenate(
        [results[k]["result_own"] for k in range(NCORES)], axis=0
    ).astype(np.float32)
    new_centers = np.concatenate(
        [results[k]["new_centers_shard"][1 : CS + 1] for k in range(NCORES)], axis=0
    ).astype(np.float32)
    return result, new_centers


def run_traced(x, onehot, centers, trace=True):
    """Run on hardware with NTFF profiling; returns ((result, new_centers), exec_ns)."""
    nc = _get_nc()
    in_maps = _make_in_maps(x, onehot, centers)
    res = bass_utils.run_bass_kernel_spmd(
        nc, in_maps, list(range(NCORES)), trace=trace
    )
    return _assemble(res.results), res.exec_time_ns


def kernel(x, onehot, centers):
    nc = _get_nc()
    in_maps = _make_in_maps(x, onehot, centers)
    res = bass_utils.run_bass_kernel_spmd(nc, in_maps, list(range(NCORES)))
    return _assemble(res.results)


# revision 29
# speedup vs baseline: 1.4856x; 1.1331x over previous
"""CenterLossLayer Trainium2 kernel (8-core SPMD).

Reference computation (B=4096 samples, C=100000 classes, D=128):
    gathered      = centers[labels]                      # via dense one-hot matmul
    delta[c]      = cnt_c * centers[c] - sum_{i: l_i=c} x_i
    new_centers   = centers - 0.5 * delta / (cnt + 1)
    result_i      = ||x_i - gathered_i||^2

Sharding: batch split 8 ways (512 samples/core) for the compute;
classes split 8 ways (12500 rows/core) for the new_centers output.
Each core:
  1. bulk-copies its centers class-shard -> output shard rows [1..CS]
     (dominant memory traffic; row 0 of the output stays all-zero)
  2. computes, for its 512 samples, the FINAL new-center row
     v_i = g - 0.5*(cnt*g - sx)/(cnt+1), where cnt/sx are segment
     counts/sums over the FULL batch obtained with an equality-matrix
     matmul on the tensor engine.  The matmul runs in bf16 with x split
     hi/lo (rhs = [x_hi | x_lo | 1]), giving f32-accurate sums and exact
     counts while using the fast bf16 PE path.
  3. AllGathers the v rows (every core then holds all 4096 rows)
  4. scatter-writes all 4096 rows into its own shard with ONE
     dma_scatter_add: this ucode computes out[idx] = out[row0] + src, so
     with row 0 kept all-zero it is an exact overwrite (verified bitwise
     on HW, including duplicate indices — duplicates carry identical
     bits).  Out-of-shard rows are clamped to a trash row.
"""

import os
import sys

import numpy as np

for _p in ("/opt/trn_rl_repo", "/root/.axon_site/_ro/trn_rl_repo"):
    if os.path.isdir(_p) and _p not in sys.path:
        sys.path.insert(0, _p)

import concourse.bass as bass
import concourse.bacc as bacc
import concourse.mybir as mybir
import concourse.tile as tile
from concourse import bass_utils

import ml_dtypes

B, C, D = 4096, 100000, 128
NCORES = 8
BS = B // NCORES          # 512 samples per core
CS = C // NCORES          # 12500 classes per core
TRASH = CS + 1            # trash row (after the zero row + CS class rows)
NB = B // 128             # 32 all-batch chunks
NM = BS // 128            # 4 own-batch chunks
XW = 2 * D + 1            # [x_hi | x_lo | ones] columns

F32 = mybir.dt.float32
BF16 = mybir.dt.bfloat16
I32 = mybir.dt.int32
I16 = mybir.dt.int16

_NC_CACHE = None


def _build_nc(dbg=False):
    nc = bacc.Bacc(
        "TRN2",
        target_bir_lowering=False,
        debug=False,
        num_devices=NCORES,
        # default 16KB ring fits only ~2 in-flight indirect DMAs; the final
        # scatter issues 32 back-to-back, so give the SWDGE ring real depth
        dynamic_dma_scratch_size=131072,
    )

    x_own = nc.dram_tensor("x_own", [BS, D], F32, kind="ExternalInput")
    xcat = nc.dram_tensor("xcat", [B, XW], BF16, kind="ExternalInput")
    lab_own = nc.dram_tensor("lab_own", [1, BS], F32, kind="ExternalInput")
    lab_all_pt = nc.dram_tensor("lab_all_pt", [128, NB], F32, kind="ExternalInput")
    gidx_pt = nc.dram_tensor("gidx_pt", [128, NM], I32, kind="ExternalInput")
    valid_pt = nc.dram_tensor("valid_pt", [128, NM], F32, kind="ExternalInput")
    soff = nc.dram_tensor("soff", [128, NB], I32, kind="ExternalInput")
    centers_all = nc.dram_tensor("centers_all", [C, D], F32, kind="ExternalInput")
    centers_shard = nc.dram_tensor("centers_shard", [CS, D], F32, kind="ExternalInput")

    result_own = nc.dram_tensor("result_own", [BS, 1], F32, kind="ExternalOutput")
    # row 0: always zero (scatter base); rows 1..CS: classes; row CS+1: trash
    out_shard = nc.dram_tensor("new_centers_shard", [CS + 2, D], F32, kind="ExternalOutput")

    v_own_d = nc.dram_tensor("v_own_d", [BS, D], F32)
    v_all_d = nc.dram_tensor("v_all_d", [B, D], F32, addr_space="Shared")

    if dbg:
        vall_dump = nc.dram_tensor("vall_dump", [B, D], F32, kind="ExternalOutput")

    with tile.TileContext(nc) as tc:
        with (
            tc.tile_pool(name="sbuf", bufs=1) as cpool,
            tc.tile_pool(name="ebuf", bufs=4) as epool,
            tc.tile_pool(name="psum", bufs=1, space="PSUM") as ppool,
        ):
            # --- stage inputs in SBUF (sync ring) ---
            lab_all_sb = cpool.tile([128, NB], F32)
            nc.sync.dma_start(out=lab_all_sb[:], in_=lab_all_pt[:, :])
            lab_own_sb = cpool.tile([1, BS], F32)
            nc.sync.dma_start(out=lab_own_sb[:], in_=lab_own[:, :])
            gidx_sb = cpool.tile([128, NM], I32)
            nc.sync.dma_start(out=gidx_sb[:], in_=gidx_pt[:, :])
            valid_sb = cpool.tile([128, NM], F32)
            nc.sync.dma_start(out=valid_sb[:], in_=valid_pt[:, :])
            soff_sb = cpool.tile([128, NB], I32)
            nc.sync.dma_start(out=soff_sb[:], in_=soff[:, :])
            xc_sb = cpool.tile([128, NB, XW], BF16)
            nc.sync.dma_start(
                out=xc_sb[:], in_=xcat.rearrange("(t p) c -> p t c", p=128)
            )
            x_own_sb = cpool.tile([128, NM, D], F32)
            nc.sync.dma_start(
                out=x_own_sb[:], in_=x_own.rearrange("(t p) d -> p t d", p=128)
            )

            # --- bulk copy centers shard -> output rows [1..CS] (scalar ring,
            # so it doesn't queue in front of the input loads above) ---
            n_pieces = 4
            rows = CS // n_pieces
            for i in range(n_pieces):
                nc.scalar.dma_start(
                    out=out_shard[1 + i * rows : 1 + (i + 1) * rows, :],
                    in_=centers_shard[i * rows : (i + 1) * rows, :],
                )

            # broadcast own labels across partitions: ones^T @ lab_own
            ones1 = cpool.tile([1, 128], F32)
            nc.vector.memset(ones1[:], 1.0)
            bc_psum = ppool.tile([128, BS], F32)
            nc.tensor.matmul(
                bc_psum[:], lhsT=ones1[:], rhs=lab_own_sb[:], start=True, stop=True
            )
            bcast_sb = cpool.tile([128, BS], F32)
            nc.vector.tensor_copy(bcast_sb[:], bc_psum[:])

            # gather own centers rows g = centers[labels_own]
            g_sb = cpool.tile([128, NM, D], F32)
            for m in range(NM):
                nc.gpsimd.indirect_dma_start(
                    out=g_sb[:, m, :],
                    out_offset=None,
                    in_=centers_all[:, :],
                    in_offset=bass.IndirectOffsetOnAxis(ap=gidx_sb[:, m : m + 1], axis=0),
                )

            # --- equality-matrix chunks + segment-sum matmuls (bf16) ---
            # psum[m][s, :] = [sum_x_hi (D) | sum_x_lo (D) | count (1)]
            ps = []
            for m in range(NM):
                ps.append(
                    ppool.tile([128, XW], F32, tag=f"ps{m}", name=f"ps{m}")
                )
            for j in range(NB):
                # f32 compare on DVE (fast path), f32->bf16 cast on Scalar/ACT
                # (mixed-dtype tensor_scalar is ~8x slower on DVE; ACT is idle)
                e_f = epool.tile([128, BS], F32, tag="ef", name="e_f")
                nc.vector.tensor_scalar(
                    out=e_f[:],
                    in0=bcast_sb[:],
                    scalar1=lab_all_sb[:, j : j + 1],
                    scalar2=None,
                    op0=mybir.AluOpType.is_equal,
                )
                e_t = epool.tile([128, BS], BF16, tag="e", name="e_t")
                nc.scalar.copy(out=e_t[:], in_=e_f[:])
                for m in range(NM):
                    nc.tensor.matmul(
                        ps[m][:],
                        lhsT=e_t[:, m * 128 : (m + 1) * 128],
                        rhs=xc_sb[:, j, :],
                        start=(j == 0),
                        stop=(j == NB - 1),
                    )

            # --- per-chunk epilogue: result + final new-center rows v ---
            v_sb = cpool.tile([128, NM, D], F32)
            for m in range(NM):
                psb = cpool.tile([128, XW], F32, tag=f"psb{m}", name=f"psb{m}")
                nc.vector.tensor_copy(psb[:], ps[m][:])
                sx = cpool.tile([128, D], F32, tag=f"sx{m}", name=f"sx{m}")
                nc.vector.tensor_tensor(
                    out=sx[:], in0=psb[:, 0:D], in1=psb[:, D : 2 * D],
                    op=mybir.AluOpType.add,
                )
                cnt = psb[:, 2 * D : XW]

                gm = cpool.tile([128, D], F32, tag=f"gm{m}", name=f"gm{m}")
                nc.vector.tensor_scalar(
                    out=gm[:],
                    in0=g_sb[:, m, :],
                    scalar1=valid_sb[:, m : m + 1],
                    scalar2=None,
                    op0=mybir.AluOpType.mult,
                )

                diff = cpool.tile([128, D], F32, tag=f"diff{m}", name=f"diff{m}")
                nc.vector.tensor_tensor(
                    out=diff[:], in0=x_own_sb[:, m, :], in1=gm[:],
                    op=mybir.AluOpType.subtract,
                )
                sq = cpool.tile([128, D], F32, tag=f"sq{m}", name=f"sq{m}")
                nc.vector.tensor_tensor(
                    out=sq[:], in0=diff[:], in1=diff[:], op=mybir.AluOpType.mult
                )
                res = cpool.tile([128, 1], F32, tag=f"res{m}", name=f"res{m}")
                nc.vector.tensor_reduce(
                    out=res[:], in_=sq[:], axis=mybir.AxisListType.X,
                    op=mybir.AluOpType.add,
                )
                nc.sync.dma_start(
                    out=result_own[m * 128 : (m + 1) * 128, :], in_=res[:]
                )

                # v = g + 0.5*(sx - cnt*g)/(cnt+1)   (the FINAL new-center row)
                den = cpool.tile([128, 1], F32, tag=f"den{m}", name=f"den{m}")
                nc.vector.tensor_scalar(
                    out=den[:], in0=cnt[:], scalar1=1.0, scalar2=None,
                    op0=mybir.AluOpType.add,
                )
                rec = cpool.tile([128, 1], F32, tag=f"rec{m}", name=f"rec{m}")
                nc.vector.reciprocal(rec[:], den[:])
                rec2 = cpool.tile([128, 1], F32, tag=f"rec2{m}", name=f"rec2{m}")
                nc.vector.tensor_scalar(
                    out=rec2[:], in0=rec[:], scalar1=0.5, scalar2=None,
                    op0=mybir.AluOpType.mult,
                )
                t1 = cpool.tile([128, D], F32, tag=f"t1{m}", name=f"t1{m}")
                nc.vector.tensor_scalar(
                    out=t1[:], in0=gm[:], scalar1=cnt[:], scalar2=None,
                    op0=mybir.AluOpType.mult,
                )
                t2 = cpool.tile([128, D], F32, tag=f"t2{m}", name=f"t2{m}")
                nc.vector.tensor_tensor(
                    out=t2[:], in0=sx[:], in1=t1[:],
                    op=mybir.AluOpType.subtract,
                )
                t3 = cpool.tile([128, D], F32, tag=f"t3{m}", name=f"t3{m}")
                nc.vector.tensor_scalar(
                    out=t3[:], in0=t2[:], scalar1=rec2[:], scalar2=None,
                    op0=mybir.AluOpType.mult,
                )
                nc.vector.tensor_tensor(
                    out=v_sb[:, m, :], in0=gm[:], in1=t3[:],
                    op=mybir.AluOpType.add,
                )

            # --- exchange v rows: every core ends up with all B rows ---
            nc.sync.dma_start(
                out=v_own_d.rearrange("(t p) d -> p t d", p=128), in_=v_sb[:]
            )
            nc.gpsimd.collective_compute(
                "AllGather",
                mybir.AluOpType.bypass,
                replica_groups=[list(range(NCORES))],
                ins=[v_own_d[:, :].opt()],
                outs=[v_all_d[:, :].opt()],
            )

            # --- scatter all B rows into own shard in ONE op ---
            v_all_sb = cpool.tile([128, NB, D], F32)
            nc.sync.dma_start(
                out=v_all_sb[:], in_=v_all_d.rearrange("(t p) d -> p t d", p=128)
            )
            if dbg:
                nc.sync.dma_start(
                    out=vall_dump.rearrange("(t p) d -> p t d", p=128),
                    in_=v_all_sb[:],
                )
            # pure-write scatter: 32 indirect DMAs of 128 rows each.  The
            # dma_scatter_add route was measured latency-bound (its CCE RMW
            # does an HBM base-read round trip per token); plain writes
            # pipeline across the SDMA engines.  Duplicate labels write
            # bitwise-identical rows, so colliding writes are benign.
            for t in range(NB):
                nc.gpsimd.indirect_dma_start(
                    out=out_shard[:, :],
                    out_offset=bass.IndirectOffsetOnAxis(
                        ap=soff_sb[:, t : t + 1], axis=0
                    ),
                    in_=v_all_sb[:, t, :],
                    in_offset=None,
                )

    nc.compile()
    return nc


def _get_nc():
    global _NC_CACHE
    if _NC_CACHE is None:
        _NC_CACHE = _build_nc()
    return _NC_CACHE


def _make_in_maps(x, onehot, centers):
    x = np.ascontiguousarray(np.asarray(x, dtype=np.float32))
    centers = np.ascontiguousarray(np.asarray(centers, dtype=np.float32))
    onehot = np.asarray(onehot)

    labels = np.argmax(onehot, axis=1).astype(np.int64)
    valid = np.asarray(onehot[np.arange(B), labels]) > 0.5
    labf = np.where(valid, labels.astype(np.float32), np.float32(-1.0)).astype(
        np.float32
    )
    gidx = np.where(valid, labels, 0).astype(np.int32)
    validf = valid.astype(np.float32)

    lab_all_pt = np.ascontiguousarray(labf.reshape(NB, 128).T)

    # x split hi/lo in bf16 + ones column: [B, 2D+1]
    xh = x.astype(ml_dtypes.bfloat16)
    xl = (x - xh.astype(np.float32)).astype(ml_dtypes.bfloat16)
    xcat = np.ones((B, XW), ml_dtypes.bfloat16)
    xcat[:, 0:D] = xh
    xcat[:, D : 2 * D] = xl

    in_maps = []
    for k in range(NCORES):
        sl = slice(k * BS, (k + 1) * BS)
        loc = labels - k * CS
        ok = valid & (loc >= 0) & (loc < CS)
        # +1: row 0 of the output shard is a reserved (zero) row
        loc32 = np.where(ok, loc + 1, TRASH).astype(np.int32)
        in_maps.append(
            {
                "x_own": x[sl],
                "xcat": xcat,
                "lab_own": np.ascontiguousarray(labf[sl].reshape(1, BS)),
                "lab_all_pt": lab_all_pt,
                "gidx_pt": np.ascontiguousarray(gidx[sl].reshape(NM, 128).T),
                "valid_pt": np.ascontiguousarray(validf[sl].reshape(NM, 128).T),
                "soff": np.ascontiguousarray(loc32.reshape(NB, 128).T),
                "centers_all": centers,
                "centers_shard": centers[k * CS : (k + 1) * CS],
            }
        )
    return in_maps


def _assemble(results):
    result = np.concatenate(
        [results[k]["result_own"] for k in range(NCORES)], axis=0
    ).astype(np.float32)
    new_centers = np.concatenate(
        [results[k]["new_centers_shard"][1 : CS + 1] for k in range(NCORES)], axis=0
    ).astype(np.float32)
    return result, new_centers


def run_traced(x, onehot, centers, trace=True):
    """Run on hardware with NTFF profiling; returns ((result, new_centers), exec_ns)."""
    nc = _get_nc()
    in_maps = _make_in_maps(x, onehot, centers)
    res = bass_utils.run_bass_kernel_spmd(
        nc, in_maps, list(range(NCORES)), trace=trace
    )
    return _assemble(res.results), res.exec_time_ns


def kernel(x, onehot, centers):
    nc = _get_nc()
    in_maps = _make_in_maps(x, onehot, centers)
    res = bass_utils.run_bass_kernel_spmd(nc, in_maps, list(range(NCORES)))
    return _assemble(res.results)


# revision 31
# speedup vs baseline: 1.5383x; 1.0354x over previous
"""CenterLossLayer Trainium2 kernel (8-core SPMD).

Reference computation (B=4096 samples, C=100000 classes, D=128):
    gathered      = centers[labels]                      # via dense one-hot matmul
    delta[c]      = cnt_c * centers[c] - sum_{i: l_i=c} x_i
    new_centers   = centers - 0.5 * delta / (cnt + 1)
    result_i      = ||x_i - gathered_i||^2

Sharding: batch split 8 ways (512 samples/core) for the compute;
classes split 8 ways (12500 rows/core) for the new_centers output.
Each core:
  1. bulk-copies its centers class-shard -> output shard rows [1..CS]
     (dominant memory traffic; row 0 of the output stays all-zero)
  2. computes, for its 512 samples, the FINAL new-center row
     v_i = g - 0.5*(cnt*g - sx)/(cnt+1), where cnt/sx are segment
     counts/sums over the FULL batch obtained with an equality-matrix
     matmul on the tensor engine.  The matmul runs in bf16 with x split
     hi/lo (rhs = [x_hi | x_lo | 1]), giving f32-accurate sums and exact
     counts while using the fast bf16 PE path.
  3. AllGathers the v rows (every core then holds all 4096 rows)
  4. scatter-writes all 4096 rows into its own shard with ONE
     dma_scatter_add: this ucode computes out[idx] = out[row0] + src, so
     with row 0 kept all-zero it is an exact overwrite (verified bitwise
     on HW, including duplicate indices — duplicates carry identical
     bits).  Out-of-shard rows are clamped to a trash row.
"""

import os
import sys

import numpy as np

for _p in ("/opt/trn_rl_repo", "/root/.axon_site/_ro/trn_rl_repo"):
    if os.path.isdir(_p) and _p not in sys.path:
        sys.path.insert(0, _p)

import concourse.bass as bass
import concourse.bacc as bacc
import concourse.mybir as mybir
import concourse.tile as tile
from concourse import bass_utils

import ml_dtypes

B, C, D = 4096, 100000, 128
NCORES = 8
BS = B // NCORES          # 512 samples per core
CS = C // NCORES          # 12500 classes per core
TRASH = CS + 1            # trash row (after the zero row + CS class rows)
NB = B // 128             # 32 all-batch chunks
NM = BS // 128            # 4 own-batch chunks
XW = 2 * D + 1            # [x_hi | x_lo | ones] columns

F32 = mybir.dt.float32
BF16 = mybir.dt.bfloat16
I32 = mybir.dt.int32
I16 = mybir.dt.int16

_NC_CACHE = None


def _build_nc(dbg=False):
    nc = bacc.Bacc(
        "TRN2",
        target_bir_lowering=False,
        debug=False,
        num_devices=NCORES,
        # default 16KB ring fits only ~2 in-flight indirect DMAs; the final
        # scatter issues 32 back-to-back, so give the SWDGE ring real depth
        dynamic_dma_scratch_size=131072,
    )

    x_own = nc.dram_tensor("x_own", [BS, D], F32, kind="ExternalInput")
    xcat = nc.dram_tensor("xcat", [B, XW], BF16, kind="ExternalInput")
    lab_own = nc.dram_tensor("lab_own", [1, BS], F32, kind="ExternalInput")
    lab_all_pt = nc.dram_tensor("lab_all_pt", [128, NB], F32, kind="ExternalInput")
    gidx_pt = nc.dram_tensor("gidx_pt", [128, NM], I32, kind="ExternalInput")
    valid_pt = nc.dram_tensor("valid_pt", [128, NM], F32, kind="ExternalInput")
    soff = nc.dram_tensor("soff", [128, NB], I32, kind="ExternalInput")
    centers_all = nc.dram_tensor("centers_all", [C, D], F32, kind="ExternalInput")
    centers_shard = nc.dram_tensor("centers_shard", [CS, D], F32, kind="ExternalInput")

    result_own = nc.dram_tensor("result_own", [BS, 1], F32, kind="ExternalOutput")
    # row 0: always zero (scatter base); rows 1..CS: classes; row CS+1: trash
    out_shard = nc.dram_tensor("new_centers_shard", [CS + 2, D], F32, kind="ExternalOutput")

    v_own_d = nc.dram_tensor("v_own_d", [BS, D], F32)
    v_all_d = nc.dram_tensor("v_all_d", [B, D], F32, addr_space="Shared")

    if dbg:
        vall_dump = nc.dram_tensor("vall_dump", [B, D], F32, kind="ExternalOutput")

    with tile.TileContext(nc) as tc:
        with (
            tc.tile_pool(name="sbuf", bufs=1) as cpool,
            tc.tile_pool(name="ebuf", bufs=4) as epool,
            tc.tile_pool(name="psum", bufs=1, space="PSUM") as ppool,
        ):
            # --- stage inputs in SBUF (sync ring) ---
            lab_all_sb = cpool.tile([128, NB], F32)
            nc.sync.dma_start(out=lab_all_sb[:], in_=lab_all_pt[:, :])
            lab_own_sb = cpool.tile([1, BS], F32)
            nc.sync.dma_start(out=lab_own_sb[:], in_=lab_own[:, :])
            gidx_sb = cpool.tile([128, NM], I32)
            nc.sync.dma_start(out=gidx_sb[:], in_=gidx_pt[:, :])
            valid_sb = cpool.tile([128, NM], F32)
            nc.sync.dma_start(out=valid_sb[:], in_=valid_pt[:, :])
            soff_sb = cpool.tile([128, NB], I32)
            nc.sync.dma_start(out=soff_sb[:], in_=soff[:, :])
            xc_sb = cpool.tile([128, NB, XW], BF16)
            nc.sync.dma_start(
                out=xc_sb[:], in_=xcat.rearrange("(t p) c -> p t c", p=128)
            )
            x_own_sb = cpool.tile([128, NM, D], F32)
            nc.sync.dma_start(
                out=x_own_sb[:], in_=x_own.rearrange("(t p) d -> p t d", p=128)
            )

            # --- bulk copy centers shard -> output rows [1..CS] (scalar ring,
            # so it doesn't queue in front of the input loads above) ---
            n_pieces = 4
            rows = CS // n_pieces
            for i in range(n_pieces):
                nc.scalar.dma_start(
                    out=out_shard[1 + i * rows : 1 + (i + 1) * rows, :],
                    in_=centers_shard[i * rows : (i + 1) * rows, :],
                )

            # broadcast own labels across partitions: ones^T @ lab_own
            ones1 = cpool.tile([1, 128], F32)
            nc.vector.memset(ones1[:], 1.0)
            bc_psum = ppool.tile([128, BS], F32)
            nc.tensor.matmul(
                bc_psum[:], lhsT=ones1[:], rhs=lab_own_sb[:], start=True, stop=True
            )
            bcast_sb = cpool.tile([128, BS], F32)
            nc.vector.tensor_copy(bcast_sb[:], bc_psum[:])

            # gather own centers rows g = centers[labels_own]
            g_sb = cpool.tile([128, NM, D], F32)
            for m in range(NM):
                nc.gpsimd.indirect_dma_start(
                    out=g_sb[:, m, :],
                    out_offset=None,
                    in_=centers_all[:, :],
                    in_offset=bass.IndirectOffsetOnAxis(ap=gidx_sb[:, m : m + 1], axis=0),
                )

            # --- equality-matrix chunks + segment-sum matmuls (bf16) ---
            # psum[m][s, :] = [sum_x_hi (D) | sum_x_lo (D) | count (1)]
            ps = []
            for m in range(NM):
                ps.append(
                    ppool.tile([128, XW], F32, tag=f"ps{m}", name=f"ps{m}")
                )
            for j in range(NB):
                # f32 compare on DVE (fast path), f32->bf16 cast on Scalar/ACT
                # (mixed-dtype tensor_scalar is ~8x slower on DVE; ACT is idle)
                e_f = epool.tile([128, BS], F32, tag="ef", name="e_f")
                nc.vector.tensor_scalar(
                    out=e_f[:],
                    in0=bcast_sb[:],
                    scalar1=lab_all_sb[:, j : j + 1],
                    scalar2=None,
                    op0=mybir.AluOpType.is_equal,
                )
                e_t = epool.tile([128, BS], BF16, tag="e", name="e_t")
                nc.scalar.copy(out=e_t[:], in_=e_f[:])
                for m in range(NM):
                    nc.tensor.matmul(
                        ps[m][:],
                        lhsT=e_t[:, m * 128 : (m + 1) * 128],
                        rhs=xc_sb[:, j, :],
                        start=(j == 0),
                        stop=(j == NB - 1),
                    )

            # --- per-chunk epilogue: result + final new-center rows v ---
            v_sb = cpool.tile([128, NM, D], F32)
            for m in range(NM):
                psb = cpool.tile([128, XW], F32, tag=f"psb{m}", name=f"psb{m}")
                nc.vector.tensor_copy(psb[:], ps[m][:])
                sx = cpool.tile([128, D], F32, tag=f"sx{m}", name=f"sx{m}")
                nc.vector.tensor_tensor(
                    out=sx[:], in0=psb[:, 0:D], in1=psb[:, D : 2 * D],
                    op=mybir.AluOpType.add,
                )
                cnt = psb[:, 2 * D : XW]

                gm = cpool.tile([128, D], F32, tag=f"gm{m}", name=f"gm{m}")
                nc.vector.tensor_scalar(
                    out=gm[:],
                    in0=g_sb[:, m, :],
                    scalar1=valid_sb[:, m : m + 1],
                    scalar2=None,
                    op0=mybir.AluOpType.mult,
                )

                diff = cpool.tile([128, D], F32, tag=f"diff{m}", name=f"diff{m}")
                nc.vector.tensor_tensor(
                    out=diff[:], in0=x_own_sb[:, m, :], in1=gm[:],
                    op=mybir.AluOpType.subtract,
                )
                sq = cpool.tile([128, D], F32, tag=f"sq{m}", name=f"sq{m}")
                nc.vector.tensor_tensor(
                    out=sq[:], in0=diff[:], in1=diff[:], op=mybir.AluOpType.mult
                )
                res = cpool.tile([128, 1], F32, tag=f"res{m}", name=f"res{m}")
                nc.vector.tensor_reduce(
                    out=res[:], in_=sq[:], axis=mybir.AxisListType.X,
                    op=mybir.AluOpType.add,
                )
                nc.sync.dma_start(
                    out=result_own[m * 128 : (m + 1) * 128, :], in_=res[:]
                )

                # v = g + 0.5*(sx - cnt*g)/(cnt+1)   (the FINAL new-center row)
                den = cpool.tile([128, 1], F32, tag=f"den{m}", name=f"den{m}")
                nc.vector.tensor_scalar(
                    out=den[:], in0=cnt[:], scalar1=1.0, scalar2=None,
                    op0=mybir.AluOpType.add,
                )
                rec = cpool.tile([128, 1], F32, tag=f"rec{m}", name=f"rec{m}")
                nc.vector.reciprocal(rec[:], den[:])
                rec2 = cpool.tile([128, 1], F32, tag=f"rec2{m}", name=f"rec2{m}")
                nc.vector.tensor_scalar(
                    out=rec2[:], in0=rec[:], scalar1=0.5, scalar2=None,
                    op0=mybir.AluOpType.mult,
                )
                t1 = cpool.tile([128, D], F32, tag=f"t1{m}", name=f"t1{m}")
                nc.vector.tensor_scalar(
                    out=t1[:], in0=gm[:], scalar1=cnt[:], scalar2=None,
                    op0=mybir.AluOpType.mult,
                )
                t2 = cpool.tile([128, D], F32, tag=f"t2{m}", name=f"t2{m}")
                nc.vector.tensor_tensor(
                    out=t2[:], in0=sx[:], in1=t1[:],
                    op=mybir.AluOpType.subtract,
                )
                t3 = cpool.tile([128, D], F32, tag=f"t3{m}", name=f"t3{m}")
                nc.vector.tensor_scalar(
                    out=t3[:], in0=t2[:], scalar1=rec2[:], scalar2=None,
                    op0=mybir.AluOpType.mult,
                )
                nc.vector.tensor_tensor(
                    out=v_sb[:, m, :], in0=gm[:], in1=t3[:],
                    op=mybir.AluOpType.add,
                )

            # --- exchange v rows: every core ends up with all B rows ---
            nc.sync.dma_start(
                out=v_own_d.rearrange("(t p) d -> p t d", p=128), in_=v_sb[:]
            )
            nc.gpsimd.collective_compute(
                "AllGather",
                mybir.AluOpType.bypass,
                replica_groups=[list(range(NCORES))],
                ins=[v_own_d[:, :].opt()],
                outs=[v_all_d[:, :].opt()],
            )

            # --- scatter all B rows into own shard in ONE op ---
            v_all_sb = cpool.tile([128, NB, D], F32)
            nc.sync.dma_start(
                out=v_all_sb[:], in_=v_all_d.rearrange("(t p) d -> p t d", p=128)
            )
            if dbg:
                nc.sync.dma_start(
                    out=vall_dump.rearrange("(t p) d -> p t d", p=128),
                    in_=v_all_sb[:],
                )
            # pure-write scatter: 32 indirect DMAs of 128 rows each.  The
            # dma_scatter_add route was measured latency-bound (its CCE RMW
            # does an HBM base-read round trip per token); plain writes
            # pipeline across the SDMA engines.  Duplicate labels write
            # bitwise-identical rows, so colliding writes are benign.
            # the 32 writes hit disjoint (or identical-content) rows, so the
            # WAW serialization Tile would impose between them is unnecessary;
            # a critical section issues them back-to-back on the Pool engine
            scat_sem = nc.alloc_semaphore("scatter_dma")
            with tc.tile_critical():
                for t in range(NB):
                    nc.gpsimd.indirect_dma_start(
                        out=out_shard[:, :],
                        out_offset=bass.IndirectOffsetOnAxis(
                            ap=soff_sb[:, t : t + 1], axis=0
                        ),
                        in_=v_all_sb[:, t, :],
                        in_offset=None,
                    ).then_inc(scat_sem, 16)
                nc.gpsimd.wait_ge(scat_sem, 16 * NB)

    nc.compile()
    return nc


def _get_nc():
    global _NC_CACHE
    if _NC_CACHE is None:
        _NC_CACHE = _build_nc()
    return _NC_CACHE


def _make_in_maps(x, onehot, centers):
    x = np.ascontiguousarray(np.asarray(x, dtype=np.float32))
    centers = np.ascontiguousarray(np.asarray(centers, dtype=np.float32))
    onehot = np.asarray(onehot)

    labels = np.argmax(onehot, axis=1).astype(np.int64)
    valid = np.asarray(onehot[np.arange(B), labels]) > 0.5
    labf = np.where(valid, labels.astype(np.float32), np.float32(-1.0)).astype(
        np.float32
    )
    gidx = np.where(valid, labels, 0).astype(np.int32)
    validf = valid.astype(np.float32)

    lab_all_pt = np.ascontiguousarray(labf.reshape(NB, 128).T)

    # x split hi/lo in bf16 + ones column: [B, 2D+1]
    xh = x.astype(ml_dtypes.bfloat16)
    xl = (x - xh.astype(np.float32)).astype(ml_dtypes.bfloat16)
    xcat = np.ones((B, XW), ml_dtypes.bfloat16)
    xcat[:, 0:D] = xh
    xcat[:, D : 2 * D] = xl

    in_maps = []
    for k in range(NCORES):
        sl = slice(k * BS, (k + 1) * BS)
        loc = labels - k * CS
        ok = valid & (loc >= 0) & (loc < CS)
        # +1: row 0 of the output shard is a reserved (zero) row
        loc32 = np.where(ok, loc + 1, TRASH).astype(np.int32)
        in_maps.append(
            {
                "x_own": x[sl],
                "xcat": xcat,
                "lab_own": np.ascontiguousarray(labf[sl].reshape(1, BS)),
                "lab_all_pt": lab_all_pt,
                "gidx_pt": np.ascontiguousarray(gidx[sl].reshape(NM, 128).T),
                "valid_pt": np.ascontiguousarray(validf[sl].reshape(NM, 128).T),
                "soff": np.ascontiguousarray(loc32.reshape(NB, 128).T),
                "centers_all": centers,
                "centers_shard": centers[k * CS : (k + 1) * CS],
            }
        )
    return in_maps


def _assemble(results):
    result = np.concatenate(
        [results[k]["result_own"] for k in range(NCORES)], axis=0
    ).astype(np.float32)
    new_centers = np.concatenate(
        [results[k]["new_centers_shard"][1 : CS + 1] for k in range(NCORES)], axis=0
    ).astype(np.float32)
    return result, new_centers


def run_traced(x, onehot, centers, trace=True):
    """Run on hardware with NTFF profiling; returns ((result, new_centers), exec_ns)."""
    nc = _get_nc()
    in_maps = _make_in_maps(x, onehot, centers)
    res = bass_utils.run_bass_kernel_spmd(
        nc, in_maps, list(range(NCORES)), trace=trace
    )
    return _assemble(res.results), res.exec_time_ns


def kernel(x, onehot, centers):
    nc = _get_nc()
    in_maps = _make_in_maps(x, onehot, centers)
    res = bass_utils.run_bass_kernel_spmd(nc, in_maps, list(range(NCORES)))
    return _assemble(res.results)


# revision 39
# speedup vs baseline: 2.1825x; 1.4188x over previous
"""CenterLossLayer Trainium2 kernel (8-core SPMD).

Reference computation (B=4096 samples, C=100000 classes, D=128):
    gathered      = centers[labels]                      # via dense one-hot matmul
    delta[c]      = cnt_c * centers[c] - sum_{i: l_i=c} x_i
    new_centers   = centers - 0.5 * delta / (cnt + 1)
    result_i      = ||x_i - gathered_i||^2

Sharding: batch split 8 ways (512 samples/core) for the compute;
classes split 8 ways (12500 rows/core) for the new_centers output.
Each core:
  1. bulk-copies its centers class-shard -> output shard rows [1..CS]
     (dominant memory traffic; row 0 of the output stays all-zero)
  2. computes, for its 512 samples, the FINAL new-center row
     v_i = g - 0.5*(cnt*g - sx)/(cnt+1), where cnt/sx are segment
     counts/sums over the FULL batch obtained with an equality-matrix
     matmul on the tensor engine.  The matmul runs in bf16 with x split
     hi/lo (rhs = [x_hi | x_lo | 1]), giving f32-accurate sums and exact
     counts while using the fast bf16 PE path.
  3. AllGathers the v rows (every core then holds all 4096 rows)
  4. scatter-writes all 4096 rows into its own shard with ONE
     dma_scatter_add: this ucode computes out[idx] = out[row0] + src, so
     with row 0 kept all-zero it is an exact overwrite (verified bitwise
     on HW, including duplicate indices — duplicates carry identical
     bits).  Out-of-shard rows are clamped to a trash row.
"""

import os
import sys

import numpy as np

for _p in ("/opt/trn_rl_repo", "/root/.axon_site/_ro/trn_rl_repo"):
    if os.path.isdir(_p) and _p not in sys.path:
        sys.path.insert(0, _p)

import concourse.bass as bass
import concourse.bacc as bacc
import concourse.mybir as mybir
import concourse.tile as tile
from concourse import bass_utils

import ml_dtypes

B, C, D = 4096, 100000, 128
NCORES = 8
BS = B // NCORES          # 512 samples per core
CS = C // NCORES          # 12500 classes per core
TRASH = CS + 1            # trash row (after the zero row + CS class rows)
NB = B // 128             # 32 all-batch chunks
NM = BS // 128            # 4 own-batch chunks
XW = 2 * D + 1            # [x_hi | x_lo | ones] columns

F32 = mybir.dt.float32
BF16 = mybir.dt.bfloat16
I32 = mybir.dt.int32
I16 = mybir.dt.int16

_NC_CACHE = {}

# compact-scatter capacity: rows-per-shard bound.  1024 is ~24 sigma above
# the ~512 expected for uniform labels; a 4096-capacity variant is compiled
# lazily if an input ever concentrates more distinct classes in one shard.
DEFAULT_CAP = 1024


def _build_nc(dbg=False, cap=DEFAULT_CAP):
    CAP = cap
    nc = bacc.Bacc(
        "TRN2",
        target_bir_lowering=False,
        debug=False,
        num_devices=NCORES,
        # default 16KB ring fits only ~2 in-flight indirect DMAs; the final
        # scatter issues 32 back-to-back, so give the SWDGE ring real depth
        dynamic_dma_scratch_size=131072,
    )

    x_own = nc.dram_tensor("x_own", [BS, D], F32, kind="ExternalInput")
    xcat = nc.dram_tensor("xcat", [B, XW], BF16, kind="ExternalInput")
    lab_own = nc.dram_tensor("lab_own", [1, BS], F32, kind="ExternalInput")
    lab_all_pt = nc.dram_tensor("lab_all_pt", [128, NB], F32, kind="ExternalInput")
    gidx_pt = nc.dram_tensor("gidx_pt", [128, NM], I32, kind="ExternalInput")
    valid_pt = nc.dram_tensor("valid_pt", [128, NM], F32, kind="ExternalInput")
    cidx = nc.dram_tensor("cidx", [128, CAP // 16], I16, kind="ExternalInput")
    csoff = nc.dram_tensor("csoff", [128, CAP // 128], I32, kind="ExternalInput")
    centers_all = nc.dram_tensor("centers_all", [C, D], F32, kind="ExternalInput")
    centers_shard = nc.dram_tensor("centers_shard", [CS, D], F32, kind="ExternalInput")

    result_own = nc.dram_tensor("result_own", [BS, 1], F32, kind="ExternalOutput")
    # row 0: always zero (scatter base); rows 1..CS: classes; row CS+1: trash
    out_shard = nc.dram_tensor("new_centers_shard", [CS + 2, D], F32, kind="ExternalOutput")

    v_own_d = nc.dram_tensor("v_own_d", [BS, D], F32)
    v_all_d = nc.dram_tensor("v_all_d", [B, D], F32, addr_space="Shared")

    if dbg:
        vall_dump = nc.dram_tensor("vall_dump", [B, D], F32, kind="ExternalOutput")

    with tile.TileContext(nc) as tc:
        with (
            tc.tile_pool(name="sbuf", bufs=1) as cpool,
            tc.tile_pool(name="ebuf", bufs=4) as epool,
            tc.tile_pool(name="psum", bufs=1, space="PSUM") as ppool,
        ):
            # --- stage inputs in SBUF (sync ring) ---
            lab_all_sb = cpool.tile([128, NB], F32)
            nc.sync.dma_start(out=lab_all_sb[:], in_=lab_all_pt[:, :])
            lab_own_sb = cpool.tile([1, BS], F32)
            nc.sync.dma_start(out=lab_own_sb[:], in_=lab_own[:, :])
            gidx_sb = cpool.tile([128, NM], I32)
            nc.sync.dma_start(out=gidx_sb[:], in_=gidx_pt[:, :])
            valid_sb = cpool.tile([128, NM], F32)
            nc.sync.dma_start(out=valid_sb[:], in_=valid_pt[:, :])
            cidx_sb = cpool.tile([128, CAP // 16], I16)
            nc.sync.dma_start(out=cidx_sb[:], in_=cidx[:, :])
            csoff_sb = cpool.tile([128, CAP // 128], I32)
            nc.sync.dma_start(out=csoff_sb[:], in_=csoff[:, :])
            xc_sb = cpool.tile([128, NB, XW], BF16)
            nc.sync.dma_start(
                out=xc_sb[:], in_=xcat.rearrange("(t p) c -> p t c", p=128)
            )
            x_own_sb = cpool.tile([128, NM, D], F32)
            nc.sync.dma_start(
                out=x_own_sb[:], in_=x_own.rearrange("(t p) d -> p t d", p=128)
            )

            # --- bulk copy centers shard -> output rows [1..CS] (scalar ring,
            # so it doesn't queue in front of the input loads above) ---
            n_pieces = 4
            rows = CS // n_pieces
            for i in range(n_pieces):
                nc.scalar.dma_start(
                    out=out_shard[1 + i * rows : 1 + (i + 1) * rows, :],
                    in_=centers_shard[i * rows : (i + 1) * rows, :],
                )

            # broadcast own labels across partitions: ones^T @ lab_own
            ones1 = cpool.tile([1, 128], F32)
            nc.vector.memset(ones1[:], 1.0)
            bc_psum = ppool.tile([128, BS], F32)
            nc.tensor.matmul(
                bc_psum[:], lhsT=ones1[:], rhs=lab_own_sb[:], start=True, stop=True
            )
            bcast_sb = cpool.tile([128, BS], F32)
            nc.vector.tensor_copy(bcast_sb[:], bc_psum[:])

            # gather own centers rows g = centers[labels_own]
            g_sb = cpool.tile([128, NM, D], F32)
            for m in range(NM):
                nc.gpsimd.indirect_dma_start(
                    out=g_sb[:, m, :],
                    out_offset=None,
                    in_=centers_all[:, :],
                    in_offset=bass.IndirectOffsetOnAxis(ap=gidx_sb[:, m : m + 1], axis=0),
                )

            # --- equality-matrix chunks + segment-sum matmuls (bf16) ---
            # psum[m][s, :] = [sum_x_hi (D) | sum_x_lo (D) | count (1)]
            ps = []
            for m in range(NM):
                ps.append(
                    ppool.tile([128, XW], F32, tag=f"ps{m}", name=f"ps{m}")
                )
            for j in range(NB):
                # f32 compare on DVE (fast path), f32->bf16 cast on Scalar/ACT
                # (mixed-dtype tensor_scalar is ~8x slower on DVE; ACT is idle)
                e_f = epool.tile([128, BS], F32, tag="ef", name="e_f")
                nc.vector.tensor_scalar(
                    out=e_f[:],
                    in0=bcast_sb[:],
                    scalar1=lab_all_sb[:, j : j + 1],
                    scalar2=None,
                    op0=mybir.AluOpType.is_equal,
                )
                e_t = epool.tile([128, BS], BF16, tag="e", name="e_t")
                nc.scalar.copy(out=e_t[:], in_=e_f[:])
                for m in range(NM):
                    nc.tensor.matmul(
                        ps[m][:],
                        lhsT=e_t[:, m * 128 : (m + 1) * 128],
                        rhs=xc_sb[:, j, :],
                        start=(j == 0),
                        stop=(j == NB - 1),
                    )

            # --- per-chunk epilogue: result + final new-center rows v ---
            v_sb = cpool.tile([128, NM, D], F32)
            for m in range(NM):
                psb = cpool.tile([128, XW], F32, tag=f"psb{m}", name=f"psb{m}")
                nc.vector.tensor_copy(psb[:], ps[m][:])
                sx = cpool.tile([128, D], F32, tag=f"sx{m}", name=f"sx{m}")
                nc.vector.tensor_tensor(
                    out=sx[:], in0=psb[:, 0:D], in1=psb[:, D : 2 * D],
                    op=mybir.AluOpType.add,
                )
                cnt = psb[:, 2 * D : XW]

                gm = cpool.tile([128, D], F32, tag=f"gm{m}", name=f"gm{m}")
                nc.vector.tensor_scalar(
                    out=gm[:],
                    in0=g_sb[:, m, :],
                    scalar1=valid_sb[:, m : m + 1],
                    scalar2=None,
                    op0=mybir.AluOpType.mult,
                )

                diff = cpool.tile([128, D], F32, tag=f"diff{m}", name=f"diff{m}")
                nc.vector.tensor_tensor(
                    out=diff[:], in0=x_own_sb[:, m, :], in1=gm[:],
                    op=mybir.AluOpType.subtract,
                )
                sq = cpool.tile([128, D], F32, tag=f"sq{m}", name=f"sq{m}")
                nc.vector.tensor_tensor(
                    out=sq[:], in0=diff[:], in1=diff[:], op=mybir.AluOpType.mult
                )
                res = cpool.tile([128, 1], F32, tag=f"res{m}", name=f"res{m}")
                nc.vector.tensor_reduce(
                    out=res[:], in_=sq[:], axis=mybir.AxisListType.X,
                    op=mybir.AluOpType.add,
                )
                nc.sync.dma_start(
                    out=result_own[m * 128 : (m + 1) * 128, :], in_=res[:]
                )

                # v = g + 0.5*(sx - cnt*g)/(cnt+1)   (the FINAL new-center row)
                den = cpool.tile([128, 1], F32, tag=f"den{m}", name=f"den{m}")
                nc.vector.tensor_scalar(
                    out=den[:], in0=cnt[:], scalar1=1.0, scalar2=None,
                    op0=mybir.AluOpType.add,
                )
                rec = cpool.tile([128, 1], F32, tag=f"rec{m}", name=f"rec{m}")
                nc.vector.reciprocal(rec[:], den[:])
                rec2 = cpool.tile([128, 1], F32, tag=f"rec2{m}", name=f"rec2{m}")
                nc.vector.tensor_scalar(
                    out=rec2[:], in0=rec[:], scalar1=0.5, scalar2=None,
                    op0=mybir.AluOpType.mult,
                )
                t1 = cpool.tile([128, D], F32, tag=f"t1{m}", name=f"t1{m}")
                nc.vector.tensor_scalar(
                    out=t1[:], in0=gm[:], scalar1=cnt[:], scalar2=None,
                    op0=mybir.AluOpType.mult,
                )
                t2 = cpool.tile([128, D], F32, tag=f"t2{m}", name=f"t2{m}")
                nc.vector.tensor_tensor(
                    out=t2[:], in0=sx[:], in1=t1[:],
                    op=mybir.AluOpType.subtract,
                )
                t3 = cpool.tile([128, D], F32, tag=f"t3{m}", name=f"t3{m}")
                nc.vector.tensor_scalar(
                    out=t3[:], in0=t2[:], scalar1=rec2[:], scalar2=None,
                    op0=mybir.AluOpType.mult,
                )
                nc.vector.tensor_tensor(
                    out=v_sb[:, m, :], in0=gm[:], in1=t3[:],
                    op=mybir.AluOpType.add,
                )

            # --- exchange v rows: every core ends up with all B rows ---
            nc.sync.dma_start(
                out=v_own_d.rearrange("(t p) d -> p t d", p=128), in_=v_sb[:]
            )
            nc.gpsimd.collective_compute(
                "AllGather",
                mybir.AluOpType.bypass,
                replica_groups=[list(range(NCORES))],
                ins=[v_own_d[:, :].opt()],
                outs=[v_all_d[:, :].opt()],
            )

            # --- compact scatter: gather only the rows whose class lands in
            # this core's shard (host-compacted indices, padded to CAP with
            # index 0), then CAP/128 pure-write indirect DMAs.  Padding
            # offsets point at the trash row.  Duplicate labels carry
            # bitwise-identical rows, so colliding writes are benign.
            vg_sb = cpool.tile([128, CAP // 128, D], F32)
            nc.gpsimd.dma_gather(
                vg_sb[:],
                v_all_d[:, :],
                cidx_sb[:],
                CAP,
                CAP,
                D,
            )
            if dbg:
                nc.sync.dma_start(
                    out=vall_dump.rearrange("(t p) d -> p t d", p=128)[
                        :, 0 : CAP // 128, :
                    ],
                    in_=vg_sb[:],
                )
            # the writes hit disjoint (or identical-content) rows, so the WAW
            # serialization Tile would impose between them is unnecessary; a
            # critical section issues them back-to-back on the Pool engine
            scat_sem = nc.alloc_semaphore("scatter_dma")
            with tc.tile_critical():
                for t in range(CAP // 128):
                    nc.gpsimd.indirect_dma_start(
                        out=out_shard[:, :],
                        out_offset=bass.IndirectOffsetOnAxis(
                            ap=csoff_sb[:, t : t + 1], axis=0
                        ),
                        in_=vg_sb[:, t, :],
                        in_offset=None,
                    ).then_inc(scat_sem, 16)
                nc.gpsimd.wait_ge(scat_sem, 16 * (CAP // 128))

    nc.compile()
    return nc


def _get_nc(cap=DEFAULT_CAP):
    if cap not in _NC_CACHE:
        _NC_CACHE[cap] = _build_nc(cap=cap)
    return _NC_CACHE[cap]


def _make_in_maps(x, onehot, centers):
    x = np.ascontiguousarray(np.asarray(x, dtype=np.float32))
    centers = np.ascontiguousarray(np.asarray(centers, dtype=np.float32))
    onehot = np.asarray(onehot)

    labels = np.argmax(onehot, axis=1).astype(np.int64)
    valid = np.asarray(onehot[np.arange(B), labels]) > 0.5
    labf = np.where(valid, labels.astype(np.float32), np.float32(-1.0)).astype(
        np.float32
    )
    gidx = np.where(valid, labels, 0).astype(np.int32)
    validf = valid.astype(np.float32)

    lab_all_pt = np.ascontiguousarray(labf.reshape(NB, 128).T)

    # x split hi/lo in bf16 + ones column: [B, 2D+1]
    xh = x.astype(ml_dtypes.bfloat16)
    xl = (x - xh.astype(np.float32)).astype(ml_dtypes.bfloat16)
    xcat = np.ones((B, XW), ml_dtypes.bfloat16)
    xcat[:, 0:D] = xh
    xcat[:, D : 2 * D] = xl

    # per-core compact scatter lists: sample indices whose label is in the
    # core's class shard, plus their local destination rows
    comp = []
    for k in range(NCORES):
        loc = labels - k * CS
        ok = valid & (loc >= 0) & (loc < CS)
        idxs_k = np.where(ok)[0]
        comp.append((idxs_k, loc[idxs_k]))
    max_count = max(len(c[0]) for c in comp)
    cap = DEFAULT_CAP if max_count <= DEFAULT_CAP else B

    in_maps = []
    for k in range(NCORES):
        sl = slice(k * BS, (k + 1) * BS)
        idxs_k, loc_k = comp[k]
        ci = np.zeros(cap, np.int16)
        ci[: len(idxs_k)] = idxs_k
        co = np.full(cap, TRASH, np.int32)
        # +1: row 0 of the output shard is a reserved (zero) row
        co[: len(loc_k)] = loc_k + 1
        # wrapped [i%16, i//16] layout, replicated to every 16-partition
        # window (each GPSIMD Q7 core reads its own window)
        cidx = np.tile(ci.reshape(cap // 16, 16).T, (8, 1)).astype(np.int16)
        in_maps.append(
            {
                "x_own": x[sl],
                "xcat": xcat,
                "lab_own": np.ascontiguousarray(labf[sl].reshape(1, BS)),
                "lab_all_pt": lab_all_pt,
                "gidx_pt": np.ascontiguousarray(gidx[sl].reshape(NM, 128).T),
                "valid_pt": np.ascontiguousarray(validf[sl].reshape(NM, 128).T),
                "cidx": cidx,
                "csoff": np.ascontiguousarray(co.reshape(cap // 128, 128).T),
                "centers_all": centers,
                "centers_shard": centers[k * CS : (k + 1) * CS],
            }
        )
    return in_maps, cap


def _assemble(results):
    result = np.concatenate(
        [results[k]["result_own"] for k in range(NCORES)], axis=0
    ).astype(np.float32)
    new_centers = np.concatenate(
        [results[k]["new_centers_shard"][1 : CS + 1] for k in range(NCORES)], axis=0
    ).astype(np.float32)
    return result, new_centers


def run_traced(x, onehot, centers, trace=True):
    """Run on hardware with NTFF profiling; returns ((result, new_centers), exec_ns)."""
    in_maps, cap = _make_in_maps(x, onehot, centers)
    nc = _get_nc(cap)
    res = bass_utils.run_bass_kernel_spmd(
        nc, in_maps, list(range(NCORES)), trace=trace
    )
    return _assemble(res.results), res.exec_time_ns


def kernel(x, onehot, centers):
    in_maps, cap = _make_in_maps(x, onehot, centers)
    nc = _get_nc(cap)
    res = bass_utils.run_bass_kernel_spmd(nc, in_maps, list(range(NCORES)))
    return _assemble(res.results)


# revision 54
# speedup vs baseline: 2.2729x; 1.0414x over previous
"""CenterLossLayer Trainium2 kernel (8-core SPMD).

Reference computation (B=4096 samples, C=100000 classes, D=128):
    gathered      = centers[labels]                      # via dense one-hot matmul
    delta[c]      = cnt_c * centers[c] - sum_{i: l_i=c} x_i
    new_centers   = centers - 0.5 * delta / (cnt + 1)
    result_i      = ||x_i - gathered_i||^2

Sharding: batch split 8 ways (512 samples/core) for the compute;
classes split 8 ways (12500 rows/core) for the new_centers output.
Each core:
  1. bulk-copies its centers class-shard -> output shard rows [1..CS]
     (dominant memory traffic; row 0 of the output stays all-zero)
  2. computes, for its 512 samples, the FINAL new-center row
     v_i = g - 0.5*(cnt*g - sx)/(cnt+1), where cnt/sx are segment
     counts/sums over the FULL batch obtained with an equality-matrix
     matmul on the tensor engine.  The matmul runs in bf16 with x split
     hi/lo (rhs = [x_hi | x_lo | 1]), giving f32-accurate sums and exact
     counts while using the fast bf16 PE path.
  3. AllGathers the v rows (every core then holds all 4096 rows)
  4. scatter-writes all 4096 rows into its own shard with ONE
     dma_scatter_add: this ucode computes out[idx] = out[row0] + src, so
     with row 0 kept all-zero it is an exact overwrite (verified bitwise
     on HW, including duplicate indices — duplicates carry identical
     bits).  Out-of-shard rows are clamped to a trash row.
"""

import os
import sys

import numpy as np

for _p in ("/opt/trn_rl_repo", "/root/.axon_site/_ro/trn_rl_repo"):
    if os.path.isdir(_p) and _p not in sys.path:
        sys.path.insert(0, _p)

import concourse.bass as bass
import concourse.bacc as bacc
import concourse.mybir as mybir
import concourse.tile as tile
from concourse import bass_utils

import ml_dtypes

B, C, D = 4096, 100000, 128
NCORES = 8
BS = B // NCORES          # 512 samples per core
CS = C // NCORES          # 12500 classes per core
TRASH = CS + 1            # trash row (after the zero row + CS class rows)
NB = B // 128             # 32 all-batch chunks
NM = BS // 128            # 4 own-batch chunks
XW = 2 * D + 1            # [x_hi | x_lo | ones] columns

F32 = mybir.dt.float32
BF16 = mybir.dt.bfloat16
I32 = mybir.dt.int32
I16 = mybir.dt.int16

_NC_CACHE = {}

# compact-scatter capacity: rows-per-shard bound.  1024 is ~24 sigma above
# the ~512 expected for uniform labels; a 4096-capacity variant is compiled
# lazily if an input ever concentrates more distinct classes in one shard.
DEFAULT_CAP = 1024


def _build_nc(dbg=False, cap=DEFAULT_CAP):
    CAP = cap
    nc = bacc.Bacc(
        "TRN2",
        target_bir_lowering=False,
        debug=False,
        num_devices=NCORES,
        # default 16KB ring fits only ~2 in-flight indirect DMAs; the final
        # scatter issues 32 back-to-back, so give the SWDGE ring real depth
        dynamic_dma_scratch_size=131072,
    )

    # _pt tensors are host-preswizzled to the on-chip [partition, ...] layout
    # so their loads are large contiguous per-partition descriptors
    x_own_pt = nc.dram_tensor("x_own_pt", [128, NM * D], F32, kind="ExternalInput")
    xcat_pt = nc.dram_tensor("xcat_pt", [128, NB * XW], BF16, kind="ExternalInput")
    lab_own = nc.dram_tensor("lab_own", [1, BS], F32, kind="ExternalInput")
    lab_all_pt = nc.dram_tensor("lab_all_pt", [128, NB], F32, kind="ExternalInput")
    gidx_pt = nc.dram_tensor("gidx_pt", [128, NM], I32, kind="ExternalInput")
    valid_pt = nc.dram_tensor("valid_pt", [128, NM], F32, kind="ExternalInput")
    cidx = nc.dram_tensor("cidx", [128, CAP // 16], I16, kind="ExternalInput")
    csoff = nc.dram_tensor("csoff", [128, CAP // 128], I32, kind="ExternalInput")
    centers_all = nc.dram_tensor("centers_all", [C, D], F32, kind="ExternalInput")
    centers_shard = nc.dram_tensor("centers_shard", [CS, D], F32, kind="ExternalInput")

    result_own = nc.dram_tensor("result_own", [BS, 1], F32, kind="ExternalOutput")
    # row 0: always zero (scatter base); rows 1..CS: classes; row CS+1: trash
    out_shard = nc.dram_tensor("new_centers_shard", [CS + 2, D], F32, kind="ExternalOutput")

    v_own_d = nc.dram_tensor("v_own_d", [BS, D], F32)
    v_all_d = nc.dram_tensor("v_all_d", [B, D], F32, addr_space="Shared")

    if dbg:
        vall_dump = nc.dram_tensor("vall_dump", [B, D], F32, kind="ExternalOutput")

    with tile.TileContext(nc) as tc:
        with (
            tc.tile_pool(name="sbuf", bufs=1) as cpool,
            tc.tile_pool(name="ebuf", bufs=4) as epool,
            tc.tile_pool(name="psum", bufs=1, space="PSUM") as ppool,
        ):
            # --- stage inputs in SBUF (sync ring) ---
            lab_all_sb = cpool.tile([128, NB], F32)
            nc.sync.dma_start(out=lab_all_sb[:], in_=lab_all_pt[:, :])
            lab_own_sb = cpool.tile([1, BS], F32)
            nc.sync.dma_start(out=lab_own_sb[:], in_=lab_own[:, :])
            gidx_sb = cpool.tile([128, NM], I32)
            nc.sync.dma_start(out=gidx_sb[:], in_=gidx_pt[:, :])
            valid_sb = cpool.tile([128, NM], F32)
            nc.sync.dma_start(out=valid_sb[:], in_=valid_pt[:, :])
            cidx_sb = cpool.tile([128, CAP // 16], I16)
            nc.sync.dma_start(out=cidx_sb[:], in_=cidx[:, :])
            csoff_sb = cpool.tile([128, CAP // 128], I32)
            nc.sync.dma_start(out=csoff_sb[:], in_=csoff[:, :])


            xc_sb = cpool.tile([128, NB, XW], BF16)
            nc.sync.dma_start(
                out=xc_sb[:], in_=xcat_pt.rearrange("p (t c) -> p t c", c=XW)
            )
            x_own_sb = cpool.tile([128, NM, D], F32)
            nc.sync.dma_start(
                out=x_own_sb[:], in_=x_own_pt.rearrange("p (t d) -> p t d", d=D)
            )

            # --- bulk copy centers shard -> output rows [1..CS] (scalar ring,
            # so it doesn't queue in front of the input loads above) ---
            n_pieces = 4
            rows = CS // n_pieces
            for i in range(n_pieces):
                nc.scalar.dma_start(
                    out=out_shard[1 + i * rows : 1 + (i + 1) * rows, :],
                    in_=centers_shard[i * rows : (i + 1) * rows, :],
                )

            # broadcast own labels across partitions: ones^T @ lab_own
            ones1 = cpool.tile([1, 128], F32)
            nc.vector.memset(ones1[:], 1.0)
            bc_psum = ppool.tile([128, BS], F32)
            nc.tensor.matmul(
                bc_psum[:], lhsT=ones1[:], rhs=lab_own_sb[:], start=True, stop=True
            )
            bcast_sb = cpool.tile([128, BS], F32)
            nc.vector.tensor_copy(bcast_sb[:], bc_psum[:])

            # gather own centers rows g = centers[labels_own]
            g_sb = cpool.tile([128, NM, D], F32)
            for m in range(NM):
                nc.gpsimd.indirect_dma_start(
                    out=g_sb[:, m, :],
                    out_offset=None,
                    in_=centers_all[:, :],
                    in_offset=bass.IndirectOffsetOnAxis(ap=gidx_sb[:, m : m + 1], axis=0),
                )

            # --- equality-matrix chunks + segment-sum matmuls (bf16) ---
            # psum[m][s, :] = [sum_x_hi (D) | sum_x_lo (D) | count (1)]
            ps = []
            for m in range(NM):
                ps.append(
                    ppool.tile([128, XW], F32, tag=f"ps{m}", name=f"ps{m}")
                )
            for j in range(NB):
                # f32 compare on DVE (fast path), f32->bf16 cast on Scalar/ACT
                # (mixed-dtype tensor_scalar is ~8x slower on DVE; ACT is idle)
                e_f = epool.tile([128, BS], F32, tag="ef", name="e_f")
                nc.vector.tensor_scalar(
                    out=e_f[:],
                    in0=bcast_sb[:],
                    scalar1=lab_all_sb[:, j : j + 1],
                    scalar2=None,
                    op0=mybir.AluOpType.is_equal,
                )
                e_t = epool.tile([128, BS], BF16, tag="e", name="e_t")
                nc.scalar.copy(out=e_t[:], in_=e_f[:])
                for m in range(NM):
                    nc.tensor.matmul(
                        ps[m][:],
                        lhsT=e_t[:, m * 128 : (m + 1) * 128],
                        rhs=xc_sb[:, j, :],
                        start=(j == 0),
                        stop=(j == NB - 1),
                    )

            # --- per-chunk epilogue: result + final new-center rows v ---
            v_sb = cpool.tile([128, NM, D], F32)
            for m in range(NM):
                psb = cpool.tile([128, XW], F32, tag=f"psb{m}", name=f"psb{m}")
                nc.vector.tensor_copy(psb[:], ps[m][:])
                sx = cpool.tile([128, D], F32, tag=f"sx{m}", name=f"sx{m}")
                nc.vector.tensor_tensor(
                    out=sx[:], in0=psb[:, 0:D], in1=psb[:, D : 2 * D],
                    op=mybir.AluOpType.add,
                )
                cnt = psb[:, 2 * D : XW]

                gm = cpool.tile([128, D], F32, tag=f"gm{m}", name=f"gm{m}")
                nc.vector.tensor_scalar(
                    out=gm[:],
                    in0=g_sb[:, m, :],
                    scalar1=valid_sb[:, m : m + 1],
                    scalar2=None,
                    op0=mybir.AluOpType.mult,
                )

                diff = cpool.tile([128, D], F32, tag=f"diff{m}", name=f"diff{m}")
                nc.vector.tensor_tensor(
                    out=diff[:], in0=x_own_sb[:, m, :], in1=gm[:],
                    op=mybir.AluOpType.subtract,
                )
                sq = cpool.tile([128, D], F32, tag=f"sq{m}", name=f"sq{m}")
                nc.vector.tensor_tensor(
                    out=sq[:], in0=diff[:], in1=diff[:], op=mybir.AluOpType.mult
                )
                res = cpool.tile([128, 1], F32, tag=f"res{m}", name=f"res{m}")
                nc.vector.tensor_reduce(
                    out=res[:], in_=sq[:], axis=mybir.AxisListType.X,
                    op=mybir.AluOpType.add,
                )
                nc.sync.dma_start(
                    out=result_own[m * 128 : (m + 1) * 128, :], in_=res[:]
                )

                # v = g + 0.5*(sx - cnt*g)/(cnt+1)   (the FINAL new-center row)
                den = cpool.tile([128, 1], F32, tag=f"den{m}", name=f"den{m}")
                nc.vector.tensor_scalar(
                    out=den[:], in0=cnt[:], scalar1=1.0, scalar2=None,
                    op0=mybir.AluOpType.add,
                )
                rec = cpool.tile([128, 1], F32, tag=f"rec{m}", name=f"rec{m}")
                nc.vector.reciprocal(rec[:], den[:])
                rec2 = cpool.tile([128, 1], F32, tag=f"rec2{m}", name=f"rec2{m}")
                nc.vector.tensor_scalar(
                    out=rec2[:], in0=rec[:], scalar1=0.5, scalar2=None,
                    op0=mybir.AluOpType.mult,
                )
                t1 = cpool.tile([128, D], F32, tag=f"t1{m}", name=f"t1{m}")
                nc.vector.tensor_scalar(
                    out=t1[:], in0=gm[:], scalar1=cnt[:], scalar2=None,
                    op0=mybir.AluOpType.mult,
                )
                t2 = cpool.tile([128, D], F32, tag=f"t2{m}", name=f"t2{m}")
                nc.vector.tensor_tensor(
                    out=t2[:], in0=sx[:], in1=t1[:],
                    op=mybir.AluOpType.subtract,
                )
                t3 = cpool.tile([128, D], F32, tag=f"t3{m}", name=f"t3{m}")
                nc.vector.tensor_scalar(
                    out=t3[:], in0=t2[:], scalar1=rec2[:], scalar2=None,
                    op0=mybir.AluOpType.mult,
                )
                nc.vector.tensor_tensor(
                    out=v_sb[:, m, :], in0=gm[:], in1=t3[:],
                    op=mybir.AluOpType.add,
                )

            # --- exchange v rows: every core ends up with all B rows ---
            for m in range(NM):
                nc.sync.dma_start(
                    out=v_own_d.rearrange("(t p) d -> p t d", p=128)[:, m, :],
                    in_=v_sb[:, m, :],
                )
            nc.gpsimd.collective_compute(
                "AllGather",
                mybir.AluOpType.bypass,
                replica_groups=[list(range(NCORES))],
                ins=[v_own_d[:, :].opt()],
                outs=[v_all_d[:, :].opt()],
            )

            # --- compact scatter: gather only the rows whose class lands in
            # this core's shard (host-compacted indices, padded to CAP with
            # index 0), then CAP/128 pure-write indirect DMAs.  Padding
            # offsets point at the trash row.  Duplicate labels carry
            # bitwise-identical rows, so colliding writes are benign.
            vg_sb = cpool.tile([128, CAP // 128, D], F32)
            nc.gpsimd.dma_gather(
                vg_sb[:],
                v_all_d[:, :],
                cidx_sb[:],
                CAP,
                CAP,
                D,
                single_packet=False,
            )
            if dbg:
                nc.sync.dma_start(
                    out=vall_dump.rearrange("(t p) d -> p t d", p=128)[
                        :, 0 : CAP // 128, :
                    ],
                    in_=vg_sb[:],
                )
            # the writes hit disjoint (or identical-content) rows, so the WAW
            # serialization Tile would impose between them is unnecessary; a
            # critical section issues them back-to-back on the Pool engine
            scat_sem = nc.alloc_semaphore("scatter_dma")
            with tc.tile_critical():
                for t in range(CAP // 128):
                    nc.gpsimd.indirect_dma_start(
                        out=out_shard[:, :],
                        out_offset=bass.IndirectOffsetOnAxis(
                            ap=csoff_sb[:, t : t + 1], axis=0
                        ),
                        in_=vg_sb[:, t, :],
                        in_offset=None,
                    ).then_inc(scat_sem, 16)
                nc.gpsimd.wait_ge(scat_sem, 16 * (CAP // 128))

    nc.compile()
    return nc


def _get_nc(cap=DEFAULT_CAP):
    if cap not in _NC_CACHE:
        _NC_CACHE[cap] = _build_nc(cap=cap)
    return _NC_CACHE[cap]


def _make_in_maps(x, onehot, centers):
    x = np.ascontiguousarray(np.asarray(x, dtype=np.float32))
    centers = np.ascontiguousarray(np.asarray(centers, dtype=np.float32))
    onehot = np.asarray(onehot)

    labels = np.argmax(onehot, axis=1).astype(np.int64)
    valid = np.asarray(onehot[np.arange(B), labels]) > 0.5
    labf = np.where(valid, labels.astype(np.float32), np.float32(-1.0)).astype(
        np.float32
    )
    gidx = np.where(valid, labels, 0).astype(np.int32)
    validf = valid.astype(np.float32)

    lab_all_pt = np.ascontiguousarray(labf.reshape(NB, 128).T)

    # x split hi/lo in bf16 + ones column: [B, 2D+1], preswizzled so that
    # row t*128+p lives at [p, t, :]
    xh = x.astype(ml_dtypes.bfloat16)
    xl = (x - xh.astype(np.float32)).astype(ml_dtypes.bfloat16)
    xcat = np.ones((B, XW), ml_dtypes.bfloat16)
    xcat[:, 0:D] = xh
    xcat[:, D : 2 * D] = xl
    xcat_pt = np.ascontiguousarray(
        xcat.reshape(NB, 128, XW).transpose(1, 0, 2)
    ).reshape(128, NB * XW)

    # per-core compact scatter lists: sample indices whose label is in the
    # core's class shard, plus their local destination rows
    comp = []
    for k in range(NCORES):
        loc = labels - k * CS
        ok = valid & (loc >= 0) & (loc < CS)
        idxs_k = np.where(ok)[0]
        comp.append((idxs_k, loc[idxs_k]))
    max_count = max(len(c[0]) for c in comp)
    cap = DEFAULT_CAP if max_count <= DEFAULT_CAP else B

    in_maps = []
    for k in range(NCORES):
        sl = slice(k * BS, (k + 1) * BS)
        idxs_k, loc_k = comp[k]
        ci = np.zeros(cap, np.int16)
        ci[: len(idxs_k)] = idxs_k
        co = np.full(cap, TRASH, np.int32)
        # +1: row 0 of the output shard is a reserved (zero) row
        co[: len(loc_k)] = loc_k + 1
        # wrapped [i%16, i//16] layout, replicated to every 16-partition
        # window (each GPSIMD Q7 core reads its own window)
        cidx = np.tile(ci.reshape(cap // 16, 16).T, (8, 1)).astype(np.int16)
        in_maps.append(
            {
                "x_own_pt": np.ascontiguousarray(
                    x[sl].reshape(NM, 128, D).transpose(1, 0, 2)
                ).reshape(128, NM * D),
                "xcat_pt": xcat_pt,
                "lab_own": np.ascontiguousarray(labf[sl].reshape(1, BS)),
                "lab_all_pt": lab_all_pt,
                "gidx_pt": np.ascontiguousarray(gidx[sl].reshape(NM, 128).T),
                "valid_pt": np.ascontiguousarray(validf[sl].reshape(NM, 128).T),
                "cidx": cidx,
                "csoff": np.ascontiguousarray(co.reshape(cap // 128, 128).T),
                "centers_all": centers,
                "centers_shard": centers[k * CS : (k + 1) * CS],
            }
        )
    return in_maps, cap


def _assemble(results):
    result = np.concatenate(
        [results[k]["result_own"] for k in range(NCORES)], axis=0
    ).astype(np.float32)
    new_centers = np.concatenate(
        [results[k]["new_centers_shard"][1 : CS + 1] for k in range(NCORES)], axis=0
    ).astype(np.float32)
    return result, new_centers


def run_traced(x, onehot, centers, trace=True):
    """Run on hardware with NTFF profiling; returns ((result, new_centers), exec_ns)."""
    in_maps, cap = _make_in_maps(x, onehot, centers)
    nc = _get_nc(cap)
    res = bass_utils.run_bass_kernel_spmd(
        nc, in_maps, list(range(NCORES)), trace=trace
    )
    return _assemble(res.results), res.exec_time_ns


def kernel(x, onehot, centers):
    in_maps, cap = _make_in_maps(x, onehot, centers)
    nc = _get_nc(cap)
    res = bass_utils.run_bass_kernel_spmd(nc, in_maps, list(range(NCORES)))
    return _assemble(res.results)
